# revision 1
# baseline (speedup 1.0000x reference)
"""Causal self-attention (B=2, T=2048, C=1024, H=16) on 8 Trainium2 cores.

Sharding: data-parallel over batch (2) x tensor-parallel over heads (4 groups
of 4 heads). Core c handles batch b = c//4, head group g = c%4 (heads 4g..4g+3).
Each core computes its qkv column slice, full causal TxT attention for its 4
heads, and a partial row-parallel projection. Host sums the 4 partial proj
outputs per batch and adds b_proj.

Device kernel layout notes:
- everything feature-major ("transposed"): qT/kT [d, t] so the PE contraction
  dims line up without any on-device transposes (host supplies x pre-transposed)
- matmuls run as float32r (FP22 mantissa truncation; full-rate streaming,
  unlike true fp32 which costs 4 cycles/row)
- softmax without max-subtraction (logits are ~N(0,1); exp is safe in fp32)
- the two heads of a pair occupy partitions 0-63 / 64-127 of the qT/kT chunk,
  so their K=64 score matmuls run concurrently in disjoint PE row quadrants
- the AV stationary operand is zero-padded to M=128 with an embedded all-ones
  column, so one matmul per head yields both the weighted values (rows 0-63 or
  64-127, matching the yT layout) and the softmax denominator row for free
- denominator rows are partition-broadcast with a K=128 all-ones matmul
  against a pre-zeroed staging tile; one full-width DVE reciprocal then covers
  both heads (walrus rejects K=1 / col-offset-64 fp32r matmuls on TRN2)
- causal masking: upper-triangle j-chunks are skipped entirely; diagonal
  chunks narrow the score/exp/AV column range and one [128,128] triangular
  multiplicative mask handles the partial strip
- norm and proj emission lag the attention loop by one step so the in-order
  PE instruction stream never stalls on DVE/ACT dependencies
"""

import os
import sys

sys.path.insert(0, "/opt/trn_rl_repo")

import numpy as np

P = 128
T = 2048
C = 1024
D = 64
HPC = 4          # heads per core
HD = HPC * D     # 256 qkv columns per core
CC = C // P      # 8 contraction chunks
TC = T // P      # 16 t-chunks of 128
IC = T // 512    # 4 i-chunks of 512

_NC = None
LAST_RESULTS = None


def _build_nc():
    import concourse.mybir as mybir
    import concourse.tile as tile
    from concourse import bacc
    from contextlib import ExitStack

    dt = mybir.dt
    f32 = dt.float32
    f32r = dt.float32r
    ALU = mybir.AluOpType
    ACTF = mybir.ActivationFunctionType

    nc = bacc.Bacc(
        "TRN2",
        target_bir_lowering=False,
        debug=False,
        enable_asserts=False,
        num_devices=8,
    )

    xT = nc.dram_tensor("xT", [C, T], f32r, kind="ExternalInput").ap()
    wq = nc.dram_tensor("wq", [C, HD], f32r, kind="ExternalInput").ap()
    wk = nc.dram_tensor("wk", [C, HD], f32r, kind="ExternalInput").ap()
    wv = nc.dram_tensor("wv", [C, HD], f32r, kind="ExternalInput").ap()
    bq = nc.dram_tensor("bq", [P, 2], f32, kind="ExternalInput").ap()
    bk = nc.dram_tensor("bk", [P, 2], f32, kind="ExternalInput").ap()
    bv = nc.dram_tensor("bv", [P, HD], f32, kind="ExternalInput").ap()
    wp = nc.dram_tensor("wp", [HD, C], f32r, kind="ExternalInput").ap()
    tri = nc.dram_tensor("tri", [P, P], f32, kind="ExternalInput").ap()
    tri2 = nc.dram_tensor("tri2", [P, 2 * P], f32, kind="ExternalInput").ap()
    onesp = nc.dram_tensor("onesp", [P, 2, P], f32r, kind="ExternalInput").ap()
    out = nc.dram_tensor("out", [T, C], f32, kind="ExternalOutput").ap()

    with tile.TileContext(nc) as tc, ExitStack() as ctx:
        persist = ctx.enter_context(tc.tile_pool(name="persist", bufs=1))
        qT_sb = persist.tile([P, 2, T], f32r, name="qT")    # [d%128, dchunk, t]
        kT_sb = persist.tile([P, 2, T], f32r, name="kT")
        v_sb = persist.tile([P, TC, 2, 2, P], f32r, name="v")  # [t%128, tchunk, hpair, hi, 128-padded d]
        yT_sb = persist.tile([P, 2, T], f32r, name="yT")
        wp_sb = persist.tile([P, 2, C], f32r, name="wps")
        tri_sb = persist.tile([P, P], f32, name="tris")
        tri2_sb = persist.tile([P, 2 * P], f32, name="tri2s")
        ones_sb = persist.tile([P, 2, P], f32r, name="ones")
        bq_sb = persist.tile([P, 2], f32, name="bqs")
        bk_sb = persist.tile([P, 2], f32, name="bks")
        bv_sb = persist.tile([P, 2, 2, D], f32, name="bvs")
        dsb = persist.tile([P, 2, 512], f32r, name="dsb")

        nc.sync.dma_start(wp_sb[:], wp.rearrange("(o p) n -> p o n", p=P))
        nc.sync.dma_start(tri_sb[:], tri)
        nc.sync.dma_start(tri2_sb[:], tri2)
        nc.sync.dma_start(bq_sb[:], bq)
        nc.sync.dma_start(bk_sb[:], bk)
        nc.sync.dma_start(bv_sb[:], bv.rearrange("p (hp hi d) -> p hp hi d", hi=2, d=D))
        nc.sync.dma_start(ones_sb[:], onesp)
        # zero the den staging tile once; each iteration only rewrites row 64 of
        # plane 0 / row 0 of plane 1, every other row must read as 0 for the
        # K=128 broadcast matmuls below
        nc.vector.tensor_scalar_mul(
            dsb[:, :, :], ones_sb[:, :, 0:1].to_broadcast([P, 2, 512]), 0.0
        )

        # ---------------- phase 1: qkv projections ----------------
        with (
            tc.tile_pool(name="ph1", bufs=1) as ph1,
            tc.tile_pool(name="ps1", bufs=4, space="PSUM") as ps1,
        ):
            xT_sb = ph1.tile([P, CC, T], f32r, name="xTs")
            wq_sb = ph1.tile([P, CC, HD], f32r, name="wqs")
            wk_sb = ph1.tile([P, CC, HD], f32r, name="wks")
            wv_sb = ph1.tile([P, CC, HD], f32r, name="wvs")
            # weights first so the first matmul group can start early; x load
            # split so several DMA queues run in parallel and the first t-half
            # (needed by the first two qT column groups) lands first
            xTr = xT.rearrange("(o p) t -> p o t", p=P)
            wqr = wq.rearrange("(o p) n -> p o n", p=P)
            nc.sync.dma_start(wq_sb[:, :, 0:P], wqr[:, :, 0:P])
            nc.sync.dma_start(wq_sb[:, :, P:HD], wqr[:, :, P:HD])
            for cc in range(CC):
                eng = nc.sync if cc % 2 == 0 else nc.gpsimd
                eng.dma_start(xT_sb[:, cc, 0:T // 2], xTr[:, cc, 0:T // 2])
            nc.gpsimd.dma_start(wk_sb[:], wk.rearrange("(o p) n -> p o n", p=P))
            nc.sync.dma_start(wv_sb[:], wv.rearrange("(o p) n -> p o n", p=P))
            for cc in range(CC):
                eng = nc.sync if cc % 2 == 0 else nc.gpsimd
                eng.dma_start(xT_sb[:, cc, T // 2:T], xTr[:, cc, T // 2:T])

            # qT / kT: [cout, t] = W.T @ x.T
            for W_s, B_s, dest in ((wq_sb, bq_sb, qT_sb), (wk_sb, bk_sb, kT_sb)):
                for co in range(2):
                    for tsl in range(4):
                        ps = ps1.tile([P, 512], f32, tag="qk")
                        for cc in range(CC):
                            nc.tensor.matmul(
                                ps[:],
                                W_s[:, cc, co * P:(co + 1) * P],
                                xT_sb[:, cc, tsl * 512:(tsl + 1) * 512],
                                start=(cc == 0),
                                stop=(cc == CC - 1),
                            )
                        nc.vector.tensor_tensor(
                            dest[:, co, tsl * 512:(tsl + 1) * 512],
                            ps[:],
                            B_s[:, co:co + 1].to_broadcast([P, 512]),
                            ALU.add,
                        )
            # v: natural [t, d] layout
            for tj in range(TC):
                ps = ps1.tile([P, HD], f32, tag="v")
                for cc in range(CC):
                    nc.tensor.matmul(
                        ps[:],
                        xT_sb[:, cc, tj * P:(tj + 1) * P],
                        wv_sb[:, cc, :],
                        start=(cc == 0),
                        stop=(cc == CC - 1),
                    )
                psv = ps[:].rearrange("p (hp hi d) -> p hp hi d", hi=2, d=D)
                # hi=0 weights: [v | 1 | 0...]; hi=1 weights: [1 | 0... | v].
                # The ones column makes the AV matmul also emit the softmax
                # denominator (row 64 for hi=0, row 0 for hi=1) for free.
                nc.vector.tensor_tensor(
                    v_sb[:, tj, :, 0, 0:D], psv[:, :, 0, :], bv_sb[:, :, 0, :], ALU.add
                )
                nc.vector.tensor_tensor(
                    v_sb[:, tj, :, 1, D:P], psv[:, :, 1, :], bv_sb[:, :, 1, :], ALU.add
                )
                # constant regions (memset cannot write f32r; mult/add by imm can)
                nc.vector.tensor_scalar(
                    v_sb[:, tj, :, 0, D:D + 1], psv[:, :, 0, 0:1], 0.0, 1.0,
                    ALU.mult, ALU.add,
                )
                nc.vector.tensor_scalar(
                    v_sb[:, tj, :, 1, 0:1], psv[:, :, 1, 0:1], 0.0, 1.0,
                    ALU.mult, ALU.add,
                )
                nc.vector.tensor_scalar_mul(
                    v_sb[:, tj, :, 0, D + 1:P], psv[:, :, 0, 0:D - 1], 0.0
                )
                nc.vector.tensor_scalar_mul(
                    v_sb[:, tj, :, 1, 1:D], psv[:, :, 1, 0:D - 1], 0.0
                )

        # ---------------- phase 2: attention + interleaved proj ----------------
        with (
            tc.tile_pool(name="ph2", bufs=3) as ph2,
            tc.tile_pool(name="ph3", bufs=3) as ph3,
            tc.tile_pool(name="ps2s", bufs=2, space="PSUM") as ps2s,
            tc.tile_pool(name="ps2a", bufs=2, space="PSUM") as ps2a,
        ):
            def emit_proj(cip):
                for tj in range(4 * cip, 4 * cip + 4):
                    ot = ph3.tile([P, C], f32, tag="ot")
                    pps = ps2s.tile([P, 2, 512], f32, tag="s")
                    for co in range(2):
                        for dc in range(2):
                            nc.tensor.matmul(
                                pps[:, co, :],
                                yT_sb[:, dc, tj * P:(tj + 1) * P],
                                wp_sb[:, dc, co * 512:(co + 1) * 512],
                                start=(dc == 0),
                                stop=(dc == 1),
                            )
                        nc.vector.tensor_copy(ot[:, co * 512:(co + 1) * 512], pps[:, co, :])
                    nc.gpsimd.dma_start(out[tj * P:(tj + 1) * P, :], ot[:])

            def emit_norm(hp, i0, av0, av1):
                # copy each head's denominator row into the pre-zeroed staging
                # tile, broadcast over partitions with a K=128 all-ones matmul,
                # stage through SBUF (frees the PSUM slot fast), reciprocal both
                # 64-row halves at once, scale into yT
                nc.vector.tensor_copy(dsb[D:D + 1, 0, :], av0[D:D + 1, :])
                nc.vector.tensor_copy(dsb[0:1, 1, :], av1[0:1, :])
                bps = ps2s.tile([P, 2, 512], f32, tag="s")
                nc.tensor.matmul(
                    bps[:, 0, :], ones_sb[:, 0, :], dsb[:, 1, :],
                    start=True, stop=True, skip_group_check=True,
                )
                nc.tensor.matmul(
                    bps[0:D, 0, :], ones_sb[:, 0, 0:D], dsb[:, 0, :],
                    start=True, stop=True, skip_group_check=True,
                )
                bsb = ph2.tile([P, 512], f32, tag="bsb")
                nc.scalar.copy(bsb[:, :], bps[:, 0, :])
                rec = ph2.tile([P, 512], f32, tag="rec")
                nc.vector.reciprocal(rec[:, :], bsb[:, :])
                nc.vector.tensor_tensor(
                    yT_sb[0:D, hp, i0:i0 + 512], av0[0:D, :], rec[0:D, :], ALU.mult
                )
                nc.vector.tensor_tensor(
                    yT_sb[D:P, hp, i0:i0 + 512], av1[D:P, :], rec[D:P, :], ALU.mult
                )

            pending = []
            for ci in range(IC):
                i0 = ci * 512
                njc = 4 * (ci + 1)
                for hp in range(2):
                    av0 = ps2a.tile([P, 512], f32, tag="av0")
                    av1 = ps2a.tile([P, 512], f32, tag="av1")

                    def emit_s(jc):
                        diag = jc >= 4 * ci
                        o = (jc - 4 * ci) if diag else 0
                        c0 = 2 * P if diag and o == 3 else o * P
                        sps = ps2s.tile([P, 2, 512], f32, tag="s")
                        for hi in range(2):
                            bp = D * hi
                            nc.tensor.matmul(
                                sps[:, hi, c0:512],
                                kT_sb[bp:bp + D, hp, jc * P:(jc + 1) * P],
                                qT_sb[bp:bp + D, hp, i0 + c0:i0 + 512],
                                start=True,
                                stop=True,
                                skip_group_check=True,
                            )
                        ex = ph2.tile([P, 2, 512], f32r, tag="ex")
                        nc.scalar.activation(
                            ex[:, :, c0:512],
                            sps[:, :, c0:512],
                            ACTF.Exp,
                            scale=float(D) ** -0.5,
                        )
                        if diag and o == 3:
                            # cols 256-383 are fully masked, 384-511 triangular
                            nc.vector.tensor_tensor(
                                ex[:, :, c0:512],
                                ex[:, :, c0:512],
                                tri2_sb[:, None, :].to_broadcast([P, 2, 2 * P]),
                                ALU.mult,
                            )
                        elif diag:
                            nc.vector.tensor_tensor(
                                ex[:, :, c0:c0 + P],
                                ex[:, :, c0:c0 + P],
                                tri_sb[:, None, :].to_broadcast([P, 2, P]),
                                ALU.mult,
                            )
                        return ex, c0

                    def emit_av(jc, ex, c0):
                        for hi, av in ((0, av0), (1, av1)):
                            nc.tensor.matmul(
                                av[:, c0:512],
                                v_sb[:, jc, hp, hi, :],
                                ex[:, hi, c0:512],
                                start=(jc == 0),
                                stop=(jc == njc - 1),
                                skip_group_check=True,
                            )

                    for jc in range(njc):
                        emit_av(jc, *emit_s(jc))
                    # norms lag one head-pair so the in-order PE stream never
                    # waits on the DVE den-row copies
                    pending.append((hp, i0, av0, av1))
                    if len(pending) > 1:
                        emit_norm(*pending.pop(0))
                if ci >= 1:
                    emit_proj(ci - 1)
            while pending:
                emit_norm(*pending.pop(0))
            emit_proj(IC - 1)
    nc.compile()
    return nc


def _get_nc():
    global _NC
    if _NC is None:
        _NC = _build_nc()
    return _NC


def kernel(x, W_qkv, b_qkv, W_proj, b_proj):
    global LAST_RESULTS
    from concourse import bass_utils

    x = np.asarray(x, dtype=np.float32)
    W_qkv = np.asarray(W_qkv, dtype=np.float32)
    b_qkv = np.asarray(b_qkv, dtype=np.float32)
    W_proj = np.asarray(W_proj, dtype=np.float32)
    b_proj = np.asarray(b_proj, dtype=np.float32)

    nc = _get_nc()
    tri = np.ascontiguousarray(np.triu(np.ones((P, P), dtype=np.float32)))
    tri2 = np.ascontiguousarray(
        np.concatenate([np.zeros((P, P), np.float32), tri], axis=1)
    )
    onesp = np.zeros((P, 2, P), dtype=np.float32)
    onesp[:, 0, :] = 1.0
    in_maps = []
    for c in range(8):
        b, g = divmod(c, 4)
        s = slice(HD * g, HD * g + HD)
        in_maps.append({
            "xT": np.ascontiguousarray(x[b].T),
            "wq": np.ascontiguousarray(W_qkv[:, s]),
            "wk": np.ascontiguousarray(W_qkv[:, C + HD * g:C + HD * g + HD]),
            "wv": np.ascontiguousarray(W_qkv[:, 2 * C + HD * g:2 * C + HD * g + HD]),
            "bq": np.ascontiguousarray(b_qkv[s].reshape(2, P).T),
            "bk": np.ascontiguousarray(b_qkv[C + HD * g:C + HD * g + HD].reshape(2, P).T),
            "bv": np.ascontiguousarray(
                np.broadcast_to(b_qkv[2 * C + HD * g:2 * C + HD * g + HD], (P, HD))
            ),
            "wp": np.ascontiguousarray(W_proj[s, :]),
            "tri": tri,
            "tri2": tri2,
            "onesp": onesp,
        })

    res = bass_utils.run_bass_kernel_spmd(nc, in_maps, core_ids=list(range(8)))
    LAST_RESULTS = res
    ys = []
    for b in range(2):
        y = res.results[4 * b]["out"].astype(np.float64)
        for g in range(1, 4):
            y = y + res.results[4 * b + g]["out"]
        ys.append((y + b_proj).astype(np.float32))
    return np.stack(ys, axis=0)



# revision 2
# speedup vs baseline: 1.4219x; 1.4219x over previous
"""Causal self-attention (B=2, T=2048, C=1024, H=16) on 8 Trainium2 cores.

Sharding: data-parallel over batch (2) x tensor-parallel over heads (4 groups
of 4 heads). Core c handles batch b = c//4, head group g = c%4 (heads 4g..4g+3).
Each core computes its qkv column slice, full causal TxT attention for its 4
heads, and a partial row-parallel projection. Host sums the 4 partial proj
outputs per batch and adds b_proj.

Device kernel layout notes (v2 — fp16 rework):
- all matmul operands are fp16. On TRN2 fp32r runs fp32_mode=HIGH (2 PE
  passes: 2 cycles/row + doubled LDWEIGHTS, and the row-bank conflict
  serializes the two K=64 head-quadrant score matmuls). fp16 is 1 cycle/row,
  enables FWL weight loads, and the hi=0/hi=1 score matmuls (stationary rows
  0-63 / 64-127) genuinely overlap. PSUM accumulation stays fp32.
- everything feature-major ("transposed"): qT/kT [d, t] so the PE contraction
  dims line up without any on-device transposes (host supplies x pre-cast to
  fp16 and pre-transposed)
- softmax without max-subtraction (logits are ~N(0,1); exp output max ~e^6
  fits fp16); exp runs on the ACT engine (~1 elem/lane/cycle, the second
  binding resource after the PE)
- the AV stationary operand is zero-padded to M=128 with an embedded all-ones
  column, so one matmul per head yields both the weighted values (rows 0-63 or
  64-127, matching the yT layout) and the softmax denominator row for free
- denominator rows are partition-broadcast with a single K=128 matmul against
  a constant selector matrix (col j reads row 64 for j<64 → den0, row 0 for
  j>=64 → den1) over a pre-zeroed staging tile; reciprocal via the custom-DVE
  reciprocal_approx_fast (~1 cycle/elem vs ~6 for exact reciprocal)
- causal masking: upper-triangle j-chunks are skipped entirely; diagonal
  chunks narrow the score/exp/AV column range and one [128,128] triangular
  multiplicative mask handles the partial strip
- scheduling: within each (i-chunk, head-pair) block the score matmul for
  chunk jc+1 issues before the AV for chunk jc, so the PE never waits on the
  ACT exp. The ACT engine needs ~853ns per chunk vs ~640ns of PE work, so
  "filler" PE work (the projection of finished i-chunks, plus deferred
  phase-1 work: qT/kT for the last t-quarter and the last 4 v chunks) is
  woven into the block at a fixed cadence. This keeps the PE densely busy —
  which also holds the HAM clock gate at 2.4 GHz (idle gaps re-throttle the
  PE array to 1.2 GHz for 3.4us+).
- DMA order: first-needed-first (wq/wk col group 0, then x t-quarter 0, ...),
  split across the sync and gpsimd trigger queues; output tiles stream out as
  fp16 as each projection tile completes.
"""

import os
import sys

sys.path.insert(0, "/opt/trn_rl_repo")

import numpy as np

P = 128
T = 2048
C = 1024
D = 64
HPC = 4          # heads per core
HD = HPC * D     # 256 qkv columns per core
CC = C // P      # 8 contraction chunks
TC = T // P      # 16 t-chunks of 128
IC = T // 512    # 4 i-chunks of 512

_NC = None
LAST_RESULTS = None


def _build_nc():
    import concourse.mybir as mybir
    import concourse.tile as tile
    from concourse import bacc
    from contextlib import ExitStack

    dt = mybir.dt
    f32 = dt.float32
    f16 = dt.float16
    ALU = mybir.AluOpType
    ACTF = mybir.ActivationFunctionType

    nc = bacc.Bacc(
        "TRN2",
        target_bir_lowering=False,
        debug=False,
        enable_asserts=False,
        num_devices=8,
    )

    xT = nc.dram_tensor("xT", [C, T], f16, kind="ExternalInput").ap()
    wq = nc.dram_tensor("wq", [C, HD], f16, kind="ExternalInput").ap()
    wk = nc.dram_tensor("wk", [C, HD], f16, kind="ExternalInput").ap()
    wv = nc.dram_tensor("wv", [C, HD], f16, kind="ExternalInput").ap()
    bq = nc.dram_tensor("bq", [P, 2], f32, kind="ExternalInput").ap()
    bk = nc.dram_tensor("bk", [P, 2], f32, kind="ExternalInput").ap()
    bv = nc.dram_tensor("bv", [P, HD], f32, kind="ExternalInput").ap()
    wp = nc.dram_tensor("wp", [HD, C], f16, kind="ExternalInput").ap()
    tri = nc.dram_tensor("tri", [P, P], f16, kind="ExternalInput").ap()
    tri2 = nc.dram_tensor("tri2", [P, 2 * P], f16, kind="ExternalInput").ap()
    sel = nc.dram_tensor("sel", [P, P], f16, kind="ExternalInput").ap()
    out = nc.dram_tensor("out", [T, C], f16, kind="ExternalOutput").ap()

    with tile.TileContext(nc) as tc, ExitStack() as ctx:
        persist = ctx.enter_context(tc.tile_pool(name="persist", bufs=1))
        qT_sb = persist.tile([P, 2, T], f16, name="qT")    # [d%128, dchunk, t]
        kT_sb = persist.tile([P, 2, T], f16, name="kT")
        v_sb = persist.tile([P, TC, 2, 2, P], f16, name="v")  # [t%128, tchunk, hpair, hi, 128-padded d]
        yT_sb = persist.tile([P, 2, T], f16, name="yT")
        wp_sb = persist.tile([P, 2, C], f16, name="wps")
        tri_sb = persist.tile([P, P], f16, name="tris")
        tri2_sb = persist.tile([P, 2 * P], f16, name="tri2s")
        sel_sb = persist.tile([P, P], f16, name="sels")
        bq_sb = persist.tile([P, 2], f32, name="bqs")
        bk_sb = persist.tile([P, 2], f32, name="bks")
        bv_sb = persist.tile([P, 2, 2, D], f32, name="bvs")
        dsb = persist.tile([P, 512], f16, name="dsb")

        # x / weight staging stays open the whole kernel: the last t-quarter of
        # qT/kT and the last 4 v chunks are emitted as PE fillers inside the
        # attention phase.
        ph1 = ctx.enter_context(tc.tile_pool(name="ph1", bufs=1))
        xT_sb = ph1.tile([P, CC, T], f16, name="xTs")
        wq_sb = ph1.tile([P, CC, HD], f16, name="wqs")
        wk_sb = ph1.tile([P, CC, HD], f16, name="wks")
        wv_sb = ph1.tile([P, CC, HD], f16, name="wvs")

        ps1 = ctx.enter_context(tc.tile_pool(name="ps1", bufs=2, space="PSUM"))
        ph2 = ctx.enter_context(tc.tile_pool(name="ph2", bufs=3))
        ph3 = ctx.enter_context(tc.tile_pool(name="ph3", bufs=3))
        ps2s = ctx.enter_context(tc.tile_pool(name="ps2s", bufs=2, space="PSUM"))
        ps2a = ctx.enter_context(tc.tile_pool(name="ps2a", bufs=1, space="PSUM"))

        # ---------------- DMA, first-needed-first ----------------
        xTr = xT.rearrange("(o p) t -> p o t", p=P)
        wqr = wq.rearrange("(o p) n -> p o n", p=P)
        wkr = wk.rearrange("(o p) n -> p o n", p=P)

        nc.sync.dma_start(bq_sb[:], bq)
        nc.sync.dma_start(tri_sb[:], tri)
        nc.sync.dma_start(tri2_sb[:], tri2)
        nc.sync.dma_start(sel_sb[:], sel)
        nc.gpsimd.dma_start(bk_sb[:], bk)
        nc.gpsimd.dma_start(
            bv_sb[:], bv.rearrange("p (hp hi d) -> p hp hi d", hi=2, d=D)
        )
        nc.sync.dma_start(wq_sb[:, :, 0:P], wqr[:, :, 0:P])
        nc.gpsimd.dma_start(wk_sb[:, :, 0:P], wkr[:, :, 0:P])
        # x quarter 0, split across both trigger queues
        nc.sync.dma_start(xT_sb[:, 0:4, 0:512], xTr[:, 0:4, 0:512])
        nc.gpsimd.dma_start(xT_sb[:, 4:8, 0:512], xTr[:, 4:8, 0:512])
        nc.sync.dma_start(wq_sb[:, :, P:HD], wqr[:, :, P:HD])
        nc.gpsimd.dma_start(wk_sb[:, :, P:HD], wkr[:, :, P:HD])
        nc.gpsimd.dma_start(wv_sb[:], wv.rearrange("(o p) n -> p o n", p=P))
        for tq in range(1, 4):
            s = slice(tq * 512, (tq + 1) * 512)
            nc.sync.dma_start(xT_sb[:, 0:4, s], xTr[:, 0:4, s])
            nc.gpsimd.dma_start(xT_sb[:, 4:8, s], xTr[:, 4:8, s])
        nc.sync.dma_start(wp_sb[:], wp.rearrange("(o p) n -> p o n", p=P))

        # zero the den staging tile once; each block only rewrites rows 0/64,
        # every other row must read as 0 for the K=128 selector matmul
        nc.vector.tensor_scalar_mul(
            dsb[:, :], sel_sb[:, 0:1].to_broadcast([P, 512]), 0.0
        )

        # ---------------- phase 1 units ----------------
        def emit_qk(W_s, B_s, dest, co, tsl):
            ps = ps1.tile([P, 512], f32, tag="qk")
            for cc in range(CC):
                nc.tensor.matmul(
                    ps[:],
                    W_s[:, cc, co * P:(co + 1) * P],
                    xT_sb[:, cc, tsl * 512:(tsl + 1) * 512],
                    start=(cc == 0),
                    stop=(cc == CC - 1),
                )
            nc.vector.tensor_tensor(
                dest[:, co, tsl * 512:(tsl + 1) * 512],
                ps[:],
                B_s[:, co:co + 1].to_broadcast([P, 512]),
                ALU.add,
            )

        def emit_v(tj):
            ps = ps1.tile([P, 512], f32, tag="qk")
            for cc in range(CC):
                nc.tensor.matmul(
                    ps[:, 0:HD],
                    xT_sb[:, cc, tj * P:(tj + 1) * P],
                    wv_sb[:, cc, :],
                    start=(cc == 0),
                    stop=(cc == CC - 1),
                )
            psv = ps[:, 0:HD].rearrange("p (hp hi d) -> p hp hi d", hi=2, d=D)
            # hi=0 weights: [v | 1 | 0...]; hi=1 weights: [1 | 0... | v].
            # The ones column makes the AV matmul also emit the softmax
            # denominator (row 64 for hi=0, row 0 for hi=1) for free.
            nc.vector.tensor_tensor(
                v_sb[:, tj, :, 0, 0:D], psv[:, :, 0, :], bv_sb[:, :, 0, :], ALU.add
            )
            nc.vector.tensor_tensor(
                v_sb[:, tj, :, 1, D:P], psv[:, :, 1, :], bv_sb[:, :, 1, :], ALU.add
            )
            nc.vector.tensor_scalar(
                v_sb[:, tj, :, 0, D:D + 1], psv[:, :, 0, 0:1], 0.0, 1.0,
                ALU.mult, ALU.add,
            )
            nc.vector.tensor_scalar(
                v_sb[:, tj, :, 1, 0:1], psv[:, :, 1, 0:1], 0.0, 1.0,
                ALU.mult, ALU.add,
            )
            nc.vector.tensor_scalar_mul(
                v_sb[:, tj, :, 0, D + 1:P], psv[:, :, 0, 0:D - 1], 0.0
            )
            nc.vector.tensor_scalar_mul(
                v_sb[:, tj, :, 1, 1:D], psv[:, :, 1, 0:D - 1], 0.0
            )

        # main phase 1: t-quarters 0-2 of qT/kT + v chunks 0-11. The rest
        # (quarter 3 + v 12-15) becomes attention-phase filler.
        for tsl in range(3):
            for W_s, B_s, dest in ((wq_sb, bq_sb, qT_sb), (wk_sb, bk_sb, kT_sb)):
                for co in range(2):
                    emit_qk(W_s, B_s, dest, co, tsl)
            for tj in range(4 * tsl, 4 * tsl + 4):
                emit_v(tj)

        fillers = []
        for co in range(2):
            fillers.append(lambda co=co: emit_qk(wk_sb, bk_sb, kT_sb, co, 3))
            fillers.append(lambda co=co: emit_qk(wq_sb, bq_sb, qT_sb, co, 3))
        for tj in range(12, 16):
            fillers.append(lambda tj=tj: emit_v(tj))

        # ---------------- phase 2: attention + woven proj ----------------
        def emit_proj(tj):
            ot = ph3.tile([P, C], f16, tag="ot")
            pps = ps2s.tile([P, 2, 512], f32, tag="s")
            for co in range(2):
                for dc in range(2):
                    nc.tensor.matmul(
                        pps[:, co, :],
                        yT_sb[:, dc, tj * P:(tj + 1) * P],
                        wp_sb[:, dc, co * 512:(co + 1) * 512],
                        start=(dc == 0),
                        stop=(dc == 1),
                    )
                nc.vector.tensor_copy(ot[:, co * 512:(co + 1) * 512], pps[:, co, :])
            eng = nc.gpsimd if tj % 2 else nc.sync
            eng.dma_start(out[tj * P:(tj + 1) * P, :], ot[:])

        def fill(n):
            for _ in range(n):
                if not fillers:
                    return
                fillers.pop(0)()

        for ci in range(IC):
            i0 = ci * 512
            njc = 4 * (ci + 1)
            # phase-1 stragglers must land before the blocks that read them
            if ci == 3:
                fill(len(fillers))
            for hp in range(2):
                av0 = ps2a.tile([P, 512], f32, tag="av0")
                av1 = ps2a.tile([P, 512], f32, tag="av1")

                def emit_s(jc):
                    diag = jc >= 4 * ci
                    o = (jc - 4 * ci) if diag else 0
                    c0 = 2 * P if diag and o == 3 else o * P
                    sps = ps2s.tile([P, 2, 512], f32, tag="s")
                    for hi in range(2):
                        bp = D * hi
                        nc.tensor.matmul(
                            sps[:, hi, c0:512],
                            kT_sb[bp:bp + D, hp, jc * P:(jc + 1) * P],
                            qT_sb[bp:bp + D, hp, i0 + c0:i0 + 512],
                            start=True,
                            stop=True,
                            skip_group_check=True,
                        )
                    ex = ph2.tile([P, 2, 512], f16, tag="ex")
                    nc.scalar.activation(
                        ex[:, :, c0:512],
                        sps[:, :, c0:512],
                        ACTF.Exp,
                        scale=float(D) ** -0.5,
                    )
                    if diag and o == 3:
                        # cols 256-383 are fully masked, 384-511 triangular
                        nc.vector.tensor_tensor(
                            ex[:, :, c0:512],
                            ex[:, :, c0:512],
                            tri2_sb[:, None, :].to_broadcast([P, 2, 2 * P]),
                            ALU.mult,
                        )
                    elif diag:
                        nc.vector.tensor_tensor(
                            ex[:, :, c0:c0 + P],
                            ex[:, :, c0:c0 + P],
                            tri_sb[:, None, :].to_broadcast([P, 2, P]),
                            ALU.mult,
                        )
                    return ex, c0

                def emit_av(jc, ex, c0):
                    for hi, av in ((0, av0), (1, av1)):
                        nc.tensor.matmul(
                            av[:, c0:512],
                            v_sb[:, jc, hp, hi, :],
                            ex[:, hi, c0:512],
                            start=(jc == 0),
                            stop=(jc == njc - 1),
                            skip_group_check=True,
                        )

                # score for jc+1 issues before AV for jc so the in-order PE
                # stream never waits on the ACT exp; fillers pad the PE to the
                # ACT rate
                pend = None
                for jc in range(njc):
                    ex, c0 = emit_s(jc)
                    if pend is not None:
                        emit_av(*pend)
                    pend = (jc, ex, c0)
                    if jc % 4 == 3:
                        fill(1)
                emit_av(*pend)

                # normalize: den rows -> staging, selector-matmul broadcast,
                # fast reciprocal, scale into yT
                nc.vector.tensor_copy(dsb[D:D + 1, :], av0[D:D + 1, :])
                nc.vector.tensor_copy(dsb[0:1, :], av1[0:1, :])
                fill(1)
                bps = ps2s.tile([P, 2, 512], f32, tag="s")
                nc.tensor.matmul(
                    bps[:, 0, :], sel_sb[:], dsb[:],
                    start=True, stop=True, skip_group_check=True,
                )
                rec = ph2.tile([P, 512], f32, tag="rec")
                nc.vector.reciprocal_approx_fast(rec[:, :], bps[:, 0, :])
                nc.vector.tensor_tensor(
                    yT_sb[0:D, hp, i0:i0 + 512], av0[0:D, :], rec[0:D, :], ALU.mult
                )
                nc.vector.tensor_tensor(
                    yT_sb[D:P, hp, i0:i0 + 512], av1[D:P, :], rec[D:P, :], ALU.mult
                )
            for tj in range(4 * ci, 4 * ci + 4):
                fillers.append(lambda tj=tj: emit_proj(tj))
        fill(len(fillers))
    nc.compile()
    return nc


def _get_nc():
    global _NC
    if _NC is None:
        _NC = _build_nc()
    return _NC


def _make_consts():
    trif = np.triu(np.ones((P, P), dtype=np.float16))
    tri2f = np.ascontiguousarray(
        np.concatenate([np.zeros((P, P), np.float16), trif], axis=1)
    )
    # selector: col j<64 reads row 64 (den0), col j>=64 reads row 0 (den1)
    self_sel = np.zeros((P, P), dtype=np.float16)
    self_sel[D, 0:D] = 1.0
    self_sel[0, D:P] = 1.0
    return np.ascontiguousarray(trif), tri2f, np.ascontiguousarray(self_sel)


def kernel(x, W_qkv, b_qkv, W_proj, b_proj):
    global LAST_RESULTS
    from concourse import bass_utils

    x = np.asarray(x, dtype=np.float32)
    W_qkv = np.asarray(W_qkv, dtype=np.float32)
    b_qkv = np.asarray(b_qkv, dtype=np.float32)
    W_proj = np.asarray(W_proj, dtype=np.float32)
    b_proj = np.asarray(b_proj, dtype=np.float32)

    nc = _get_nc()
    tri, tri2, sel = _make_consts()
    xT16 = [np.ascontiguousarray(x[b].T.astype(np.float16)) for b in range(2)]
    in_maps = []
    for c in range(8):
        b, g = divmod(c, 4)
        s = slice(HD * g, HD * g + HD)
        in_maps.append({
            "xT": xT16[b],
            "wq": np.ascontiguousarray(W_qkv[:, s].astype(np.float16)),
            "wk": np.ascontiguousarray(
                W_qkv[:, C + HD * g:C + HD * g + HD].astype(np.float16)
            ),
            "wv": np.ascontiguousarray(
                W_qkv[:, 2 * C + HD * g:2 * C + HD * g + HD].astype(np.float16)
            ),
            "bq": np.ascontiguousarray(b_qkv[s].reshape(2, P).T),
            "bk": np.ascontiguousarray(
                b_qkv[C + HD * g:C + HD * g + HD].reshape(2, P).T
            ),
            "bv": np.ascontiguousarray(
                np.broadcast_to(b_qkv[2 * C + HD * g:2 * C + HD * g + HD], (P, HD))
            ),
            "wp": np.ascontiguousarray(W_proj[s, :].astype(np.float16)),
            "tri": tri,
            "tri2": tri2,
            "sel": sel,
        })

    res = bass_utils.run_bass_kernel_spmd(nc, in_maps, core_ids=list(range(8)))
    LAST_RESULTS = res
    ys = []
    for b in range(2):
        y = res.results[4 * b]["out"].astype(np.float64)
        for g in range(1, 4):
            y = y + res.results[4 * b + g]["out"]
        ys.append((y + b_proj).astype(np.float32))
    return np.stack(ys, axis=0)


# revision 4
# speedup vs baseline: 1.6032x; 1.1275x over previous
"""Causal self-attention (B=2, T=2048, C=1024, H=16) on 8 Trainium2 cores.

Sharding: data-parallel over batch (2) x tensor-parallel over heads (4 groups
of 4 heads). Core c handles batch b = c//4, head group g = c%4 (heads 4g..4g+3).
Each core computes its qkv column slice, full causal TxT attention for its 4
heads, and a partial row-parallel projection. Host sums the 4 partial proj
outputs per batch and adds b_proj.

Device kernel layout notes (v3):
- all matmul operands are fp16: on TRN2 fp32r runs fp32_mode=HIGH (2 PE
  passes -> 2 cycles/row, doubled LDWEIGHTS, and the row-bank conflict
  serializes the two K=64 head-quadrant score matmuls). fp16 is 1 cycle/row,
  enables FWL weight loads, and the hi=0/hi=1 score matmuls (stationary rows
  0-63 / 64-127) genuinely overlap (measured dstart ~4ns). PSUM stays fp32.
- feature-major ("transposed") layouts throughout: qT/kT [d, t] so PE
  contraction dims line up with no on-device transposes
- host pre-packs every DRAM tensor so each DMA moves 2-8KB contiguous
  per-partition lines (small strided lines measured ~85GB/s/queue; packed
  ~200+GB/s), and the first-needed tensors (wq col group 0, x t-quarter 0)
  are triggered first; all small constants ship as one [128, 516] f16 blob
- softmax without max-subtraction (logits ~N(0,1), exp fits fp16); exp runs
  on the ACT engine - the second binding resource (~70us) after the PE
  (~100us); the AV stationary operand embeds an all-ones column so each AV
  matmul also emits the softmax denominator row for free
- denominator rows are partition-broadcast with a single K=128 matmul against
  a constant selector matrix (col j<64 reads row 64 = den0, col j>=64 reads
  row 0 = den1) over a pre-zeroed staging row-pair; reciprocal via the
  custom-DVE reciprocal_approx_fast (~5x faster than exact reciprocal)
- causal masking: upper-triangle j-chunks skipped; diagonal chunks narrow the
  score/exp/AV column range to [o*128, 512) and one [128,128] triangular
  multiplicative mask handles the partial strip
- scheduling: the score matmul for chunk jc+1 issues before the AV for jc so
  the in-order PE stream never waits on ACT; "filler" PE work (projection of
  finished i-chunks + deferred phase-1 work: the last t-quarter of qT/kT and
  v chunks 12-15) is woven in at a fixed cadence to keep the PE at the ACT
  rate. Dense PE occupancy also parks the HAM clock gate at 2.4 GHz (any
  ~3.4us idle window re-throttles the PE to 1.2 GHz).
- normalization lags one block: den-row copies (DVE) issue right after the
  block's last AV, the broadcast-matmul + reciprocal + yT scale are emitted
  inside the NEXT block so the PE never stalls on DVE latency
- PSUM budget (8 banks): "s" tag [128,2,512] bufs=2 (scores, proj, den
  broadcast, and phase-1 qkv groups all share it) + av0/av1 bufs=2 each
- psum->sbuf copies for the projection ride the GpSimd engine (DVE is the
  #3 resource); output tiles stream out per-tile as fp16 on the sync queue
"""

import os
import sys

sys.path.insert(0, "/opt/trn_rl_repo")

import numpy as np

P = 128
T = 2048
C = 1024
D = 64
HPC = 4          # heads per core
HD = HPC * D     # 256 qkv columns per core
CC = C // P      # 8 contraction chunks
TC = T // P      # 16 t-chunks of 128
IC = T // 512    # 4 i-chunks of 512

# const blob column offsets
OFF_TRI = 0
OFF_SEL = 128
OFF_BQ = 256
OFF_BK = 258
OFF_BV = 260
CSTW = 516

_NC = None
LAST_RESULTS = None


def _build_nc():
    import concourse.mybir as mybir
    import concourse.tile as tile
    from concourse import bacc
    from contextlib import ExitStack

    dt = mybir.dt
    f32 = dt.float32
    f16 = dt.float16
    ALU = mybir.AluOpType
    ACTF = mybir.ActivationFunctionType

    nc = bacc.Bacc(
        "TRN2",
        target_bir_lowering=False,
        debug=False,
        enable_asserts=False,
        num_devices=8,
    )

    # host-packed layouts: contiguous per-partition lines per transfer
    xq = nc.dram_tensor("xq", [P, 4, CC, 512], f16, kind="ExternalInput").ap()
    wq2 = nc.dram_tensor("wq2", [P, 2, CC, P], f16, kind="ExternalInput").ap()
    wk2 = nc.dram_tensor("wk2", [P, 2, CC, P], f16, kind="ExternalInput").ap()
    wv2 = nc.dram_tensor("wv2", [P, CC, HD], f16, kind="ExternalInput").ap()
    wp2 = nc.dram_tensor("wp2", [P, 2, C], f16, kind="ExternalInput").ap()
    cst = nc.dram_tensor("cst", [P, CSTW], f16, kind="ExternalInput").ap()
    out = nc.dram_tensor("out", [T, C], f16, kind="ExternalOutput").ap()

    with tile.TileContext(nc) as tc, ExitStack() as ctx:
        persist = ctx.enter_context(tc.tile_pool(name="persist", bufs=1))
        qT_sb = persist.tile([P, 2, T], f16, name="qT")    # [d%128, dchunk, t]
        kT_sb = persist.tile([P, 2, T], f16, name="kT")
        v_sb = persist.tile([P, TC, 2, 2, P], f16, name="v")  # [t%128, tchunk, hpair, hi, 128-padded d]
        yT_sb = persist.tile([P, 2, T], f16, name="yT")
        wp_sb = persist.tile([P, 2, C], f16, name="wps")
        cst_sb = persist.tile([P, CSTW], f16, name="csts")
        dsb = persist.tile([P, 512], f16, name="dsb")
        xT_sb = persist.tile([P, CC, T], f16, name="xTs")
        wq_sb = persist.tile([P, CC, HD], f16, name="wqs")
        wk_sb = persist.tile([P, CC, HD], f16, name="wks")
        wv_sb = persist.tile([P, CC, HD], f16, name="wvs")

        tri_v = cst_sb[:, OFF_TRI:OFF_TRI + P]
        sel_v = cst_sb[:, OFF_SEL:OFF_SEL + P]
        bq_v = cst_sb[:, OFF_BQ:OFF_BQ + 2]
        bk_v = cst_sb[:, OFF_BK:OFF_BK + 2]
        bv_v = cst_sb[:, OFF_BV:OFF_BV + HD].rearrange(
            "p (hp hi d) -> p hp hi d", hi=2, d=D
        )

        ph2 = ctx.enter_context(tc.tile_pool(name="ph2", bufs=3))
        ph3 = ctx.enter_context(tc.tile_pool(name="ph3", bufs=3))
        ps2s = ctx.enter_context(tc.tile_pool(name="ps2s", bufs=2, space="PSUM"))
        ps2a = ctx.enter_context(tc.tile_pool(name="ps2a", bufs=2, space="PSUM"))

        # one-time inits (no inputs needed)
        nc.vector.memset(dsb[:, :], 0.0)
        # constant regions of the padded v operand: zeros + the ones column
        # that makes each AV matmul emit the softmax denominator row
        nc.vector.memset(v_sb[:, :, :, 0, D + 1:P], 0.0)
        nc.vector.memset(v_sb[:, :, :, 1, 1:D], 0.0)
        nc.vector.memset(v_sb[:, :, :, 0, D:D + 1], 1.0)
        nc.vector.memset(v_sb[:, :, :, 1, 0:1], 1.0)

        # ---------------- DMA, first-needed-first ----------------
        nc.sync.dma_start(wq_sb[:, :, 0:P], wq2[:, 0, :, :])
        nc.gpsimd.dma_start(wk_sb[:, :, 0:P], wk2[:, 0, :, :])
        nc.sync.dma_start(xT_sb[:, 0:4, 0:512], xq[:, 0, 0:4, :])
        nc.gpsimd.dma_start(xT_sb[:, 4:8, 0:512], xq[:, 0, 4:8, :])
        nc.sync.dma_start(cst_sb[:], cst)
        nc.gpsimd.dma_start(wv_sb[:], wv2)
        nc.sync.dma_start(wq_sb[:, :, P:HD], wq2[:, 1, :, :])
        nc.gpsimd.dma_start(wk_sb[:, :, P:HD], wk2[:, 1, :, :])
        for tq in range(1, 4):
            s = slice(tq * 512, (tq + 1) * 512)
            nc.sync.dma_start(xT_sb[:, 0:4, s], xq[:, tq, 0:4, :])
            nc.gpsimd.dma_start(xT_sb[:, 4:8, s], xq[:, tq, 4:8, :])
        nc.gpsimd.dma_start(wp_sb[:], wp2)

        # ---------------- phase 1 units ----------------
        def emit_qk(W_s, bco, dest, co, tsl):
            ps = ps2s.tile([P, 2, 512], f32, tag="s")
            for cc in range(CC):
                nc.tensor.matmul(
                    ps[:, 0, :],
                    W_s[:, cc, co * P:(co + 1) * P],
                    xT_sb[:, cc, tsl * 512:(tsl + 1) * 512],
                    start=(cc == 0),
                    stop=(cc == CC - 1),
                )
            nc.vector.tensor_tensor(
                dest[:, co, tsl * 512:(tsl + 1) * 512],
                ps[:, 0, :],
                bco.to_broadcast([P, 512]),
                ALU.add,
            )

        def emit_v(tj):
            ps = ps2s.tile([P, 2, 512], f32, tag="s")
            for cc in range(CC):
                nc.tensor.matmul(
                    ps[:, 0, 0:HD],
                    xT_sb[:, cc, tj * P:(tj + 1) * P],
                    wv_sb[:, cc, :],
                    start=(cc == 0),
                    stop=(cc == CC - 1),
                )
            psv = ps[:, 0, 0:HD].rearrange("p (hp hi d) -> p hp hi d", hi=2, d=D)
            nc.vector.tensor_tensor(
                v_sb[:, tj, :, 0, 0:D], psv[:, :, 0, :], bv_v[:, :, 0, :], ALU.add
            )
            nc.vector.tensor_tensor(
                v_sb[:, tj, :, 1, D:P], psv[:, :, 1, :], bv_v[:, :, 1, :], ALU.add
            )

        # main phase 1: t-quarters 0-2 of qT/kT + v chunks 0-11. The rest
        # (quarter 3 + v 12-15) becomes attention-phase PE filler.
        for tsl in range(3):
            for W_s, boff, dest in (
                (wq_sb, OFF_BQ, qT_sb),
                (wk_sb, OFF_BK, kT_sb),
            ):
                for co in range(2):
                    emit_qk(
                        W_s, cst_sb[:, boff + co:boff + co + 1], dest, co, tsl
                    )
            for tj in range(4 * tsl, 4 * tsl + 4):
                emit_v(tj)

        ph1_fill = []
        for co in range(2):
            ph1_fill.append(
                lambda co=co: emit_qk(
                    wk_sb, cst_sb[:, OFF_BK + co:OFF_BK + co + 1], kT_sb, co, 3
                )
            )
            ph1_fill.append(
                lambda co=co: emit_qk(
                    wq_sb, cst_sb[:, OFF_BQ + co:OFF_BQ + co + 1], qT_sb, co, 3
                )
            )
        for tj in range(12, 16):
            ph1_fill.append(lambda tj=tj: emit_v(tj))
        proj_fill = []

        # ---------------- phase 2: attention + woven proj ----------------
        def emit_proj(tj):
            ot = ph3.tile([P, C], f16, tag="ot")
            pps = ps2s.tile([P, 2, 512], f32, tag="s")
            for co in range(2):
                for dc in range(2):
                    nc.tensor.matmul(
                        pps[:, co, :],
                        yT_sb[:, dc, tj * P:(tj + 1) * P],
                        wp_sb[:, dc, co * 512:(co + 1) * 512],
                        start=(dc == 0),
                        stop=(dc == 1),
                    )
                nc.vector.tensor_copy(ot[:, co * 512:(co + 1) * 512], pps[:, co, :])
            nc.sync.dma_start(out[tj * P:(tj + 1) * P, :], ot[:])

        def fill(n):
            for _ in range(n):
                if ph1_fill:
                    ph1_fill.pop(0)()
                elif proj_fill:
                    proj_fill.pop(0)()
                else:
                    return

        def emit_norm(hp, i0, av0, av1):
            # den rows were already copied into dsb right after the block's
            # last AV; broadcast + reciprocal + scale into yT
            bps = ps2s.tile([P, 2, 512], f32, tag="s")
            nc.tensor.matmul(
                bps[:, 0, :], sel_v, dsb[:],
                start=True, stop=True, skip_group_check=True,
            )
            rec = ph2.tile([P, 512], f32, tag="rec")
            nc.vector.reciprocal_approx_fast(rec[:, :], bps[:, 0, :])
            nc.vector.tensor_tensor(
                yT_sb[0:D, hp, i0:i0 + 512], av0[0:D, :], rec[0:D, :], ALU.mult
            )
            nc.vector.tensor_tensor(
                yT_sb[D:P, hp, i0:i0 + 512], av1[D:P, :], rec[D:P, :], ALU.mult
            )
            if hp == 1:
                ci = i0 // 512
                for tj in range(4 * ci, 4 * ci + 4):
                    proj_fill.append(lambda tj=tj: emit_proj(tj))

        pending = None
        for ci in range(IC):
            i0 = ci * 512
            njc = 4 * (ci + 1)
            if ci == 3:
                # phase-1 stragglers must land before the blocks that read them
                while ph1_fill:
                    ph1_fill.pop(0)()
            for hp in range(2):
                av0 = ps2a.tile([P, 512], f32, tag="av0")
                av1 = ps2a.tile([P, 512], f32, tag="av1")

                def emit_s(jc):
                    diag = jc >= 4 * ci
                    o = (jc - 4 * ci) if diag else 0
                    c0 = o * P
                    sps = ps2s.tile([P, 2, 512], f32, tag="s")
                    for hi in range(2):
                        bp = D * hi
                        nc.tensor.matmul(
                            sps[:, hi, c0:512],
                            kT_sb[bp:bp + D, hp, jc * P:(jc + 1) * P],
                            qT_sb[bp:bp + D, hp, i0 + c0:i0 + 512],
                            start=True,
                            stop=True,
                            skip_group_check=True,
                        )
                    ex = ph2.tile([P, 2, 512], f16, tag="ex")
                    nc.scalar.activation(
                        ex[:, :, c0:512],
                        sps[:, :, c0:512],
                        ACTF.Exp,
                        scale=float(D) ** -0.5,
                    )
                    if diag:
                        nc.vector.tensor_tensor(
                            ex[:, :, c0:c0 + P],
                            ex[:, :, c0:c0 + P],
                            tri_v[:, None, :].to_broadcast([P, 2, P]),
                            ALU.mult,
                        )
                    return ex, c0

                def emit_av(jc, ex, c0):
                    for hi, av in ((0, av0), (1, av1)):
                        nc.tensor.matmul(
                            av[:, c0:512],
                            v_sb[:, jc, hp, hi, :],
                            ex[:, hi, c0:512],
                            start=(jc == 0),
                            stop=(jc == njc - 1),
                            skip_group_check=True,
                        )

                # score jc+1 issues before AV jc so the in-order PE stream
                # never waits on the ACT exp; fillers pad the PE to ACT rate;
                # the previous block's normalization lands at jc==2
                pend_av = None
                for jc in range(njc):
                    ex, c0 = emit_s(jc)
                    if pend_av is not None:
                        emit_av(*pend_av)
                    pend_av = (jc, ex, c0)
                    if jc == 2 and pending is not None:
                        emit_norm(*pending)
                        pending = None
                    if jc % 4 == 3:
                        fill(1)
                    if jc == njc - 1 and njc >= 12:
                        fill(1)
                emit_av(*pend_av)
                # den rows -> staging now; the rest of the normalization is
                # emitted inside the next block
                nc.vector.tensor_copy(dsb[D:D + 1, :], av0[D:D + 1, :])
                nc.vector.tensor_copy(dsb[0:1, :], av1[0:1, :])
                pending = (hp, i0, av0, av1)
        fill(1)
        emit_norm(*pending)
        while proj_fill:
            proj_fill.pop(0)()
    nc.compile()
    return nc


def _get_nc():
    global _NC
    if _NC is None:
        _NC = _build_nc()
    return _NC


def _pack_inputs(x_b, W_qkv, b_qkv, W_proj, g):
    """Host-side packing for core (batch, head-group g): fp16, DMA-friendly."""
    f16 = np.float16
    s0 = HD * g
    xt = np.ascontiguousarray(x_b.T).astype(f16)          # [C, T]
    xqa = np.ascontiguousarray(
        xt.reshape(CC, P, 4, 512).transpose(1, 2, 0, 3)   # [p, quarter, o, t]
    )

    def wpack(col0):
        w = W_qkv[:, col0:col0 + HD].astype(f16)          # [C, HD]
        return np.ascontiguousarray(w.reshape(CC, P, 2, P).transpose(1, 2, 0, 3))

    wv_ = W_qkv[:, 2 * C + s0:2 * C + s0 + HD].astype(f16)
    wv_p = np.ascontiguousarray(wv_.reshape(CC, P, HD).transpose(1, 0, 2))
    wp_ = W_proj[s0:s0 + HD, :].astype(f16)               # [HD, C]
    wp_p = np.ascontiguousarray(wp_.reshape(2, P, C).transpose(1, 0, 2))

    cstm = np.zeros((P, CSTW), dtype=f16)
    cstm[:, OFF_TRI:OFF_TRI + P] = np.triu(np.ones((P, P), dtype=f16))
    cstm[D, OFF_SEL:OFF_SEL + D] = 1.0
    cstm[0, OFF_SEL + D:OFF_SEL + P] = 1.0
    cstm[:, OFF_BQ:OFF_BQ + 2] = b_qkv[s0:s0 + HD].reshape(2, P).T
    cstm[:, OFF_BK:OFF_BK + 2] = (
        b_qkv[C + s0:C + s0 + HD].reshape(2, P).T
    )
    cstm[:, OFF_BV:OFF_BV + HD] = b_qkv[2 * C + s0:2 * C + s0 + HD]

    return {
        "xq": xqa,
        "wq2": wpack(s0),
        "wk2": wpack(C + s0),
        "wv2": wv_p,
        "wp2": wp_p,
        "cst": np.ascontiguousarray(cstm),
    }


def kernel(x, W_qkv, b_qkv, W_proj, b_proj):
    global LAST_RESULTS
    from concourse import bass_utils

    x = np.asarray(x, dtype=np.float32)
    W_qkv = np.asarray(W_qkv, dtype=np.float32)
    b_qkv = np.asarray(b_qkv, dtype=np.float32)
    W_proj = np.asarray(W_proj, dtype=np.float32)
    b_proj = np.asarray(b_proj, dtype=np.float32)

    nc = _get_nc()
    in_maps = []
    for c in range(8):
        b, g = divmod(c, 4)
        in_maps.append(_pack_inputs(x[b], W_qkv, b_qkv, W_proj, g))

    res = bass_utils.run_bass_kernel_spmd(nc, in_maps, core_ids=list(range(8)))
    LAST_RESULTS = res
    ys = []
    for b in range(2):
        y = res.results[4 * b]["out"].astype(np.float64)
        for g in range(1, 4):
            y = y + res.results[4 * b + g]["out"]
        ys.append((y + b_proj).astype(np.float32))
    return np.stack(ys, axis=0)


# revision 9
# speedup vs baseline: 1.6088x; 1.0035x over previous
"""Causal self-attention (B=2, T=2048, C=1024, H=16) on 8 Trainium2 cores.

Sharding: data-parallel over batch (2) x tensor-parallel over heads (4 groups
of 4 heads). Core c handles batch b = c//4, head group g = c%4 (heads 4g..4g+3).
Each core computes its qkv column slice, full causal TxT attention for its 4
heads, and a partial row-parallel projection. Host sums the 4 partial proj
outputs per batch and adds b_proj.

Device kernel layout notes (v3):
- all matmul operands are fp16: on TRN2 fp32r runs fp32_mode=HIGH (2 PE
  passes -> 2 cycles/row, doubled LDWEIGHTS, and the row-bank conflict
  serializes the two K=64 head-quadrant score matmuls). fp16 is 1 cycle/row,
  enables FWL weight loads, and the hi=0/hi=1 score matmuls (stationary rows
  0-63 / 64-127) genuinely overlap (measured dstart ~4ns). PSUM stays fp32.
- feature-major ("transposed") layouts throughout: qT/kT [d, t] so PE
  contraction dims line up with no on-device transposes
- host pre-packs every DRAM tensor so each DMA moves 2-8KB contiguous
  per-partition lines (small strided lines measured ~85GB/s/queue; packed
  ~200+GB/s), and the first-needed tensors (wq col group 0, x t-quarter 0)
  are triggered first; all small constants ship as one [128, 516] f16 blob
- softmax without max-subtraction (logits ~N(0,1), exp fits fp16); exp runs
  on the ACT engine - the second binding resource (~70us) after the PE
  (~100us); the AV stationary operand embeds an all-ones column so each AV
  matmul also emits the softmax denominator row for free
- denominator rows are partition-broadcast with a single K=128 matmul against
  a constant selector matrix (col j<64 reads row 64 = den0, col j>=64 reads
  row 0 = den1) over a pre-zeroed staging row-pair; reciprocal via the
  custom-DVE reciprocal_approx_fast (~5x faster than exact reciprocal)
- causal masking: upper-triangle j-chunks skipped; diagonal chunks narrow the
  score/exp/AV column range to [o*128, 512) and one [128,128] triangular
  multiplicative mask handles the partial strip
- scheduling: the score matmul for chunk jc+1 issues before the AV for jc so
  the in-order PE stream never waits on ACT; "filler" PE work (projection of
  finished i-chunks + deferred phase-1 work: the last t-quarter of qT/kT and
  v chunks 12-15) is woven in at a fixed cadence to keep the PE at the ACT
  rate. Dense PE occupancy also parks the HAM clock gate at 2.4 GHz (any
  ~3.4us idle window re-throttles the PE to 1.2 GHz).
- normalization lags one block: den-row copies (DVE) issue right after the
  block's last AV, the broadcast-matmul + reciprocal + yT scale are emitted
  inside the NEXT block so the PE never stalls on DVE latency
- PSUM budget (8 banks): "s" tag [128,2,512] bufs=2 (scores, proj, den
  broadcast, and phase-1 qkv groups all share it) + av0/av1 bufs=2 each
- psum->sbuf copies for the projection ride the GpSimd engine (DVE is the
  #3 resource); output tiles stream out per-tile as fp16 on the sync queue
"""

import os
import sys

sys.path.insert(0, "/opt/trn_rl_repo")

import numpy as np

P = 128
T = 2048
C = 1024
D = 64
HPC = 4          # heads per core
HD = HPC * D     # 256 qkv columns per core
CC = C // P      # 8 contraction chunks
TC = T // P      # 16 t-chunks of 128
IC = T // 512    # 4 i-chunks of 512

# const blob column offsets
OFF_TRI = 0
OFF_SEL = 128
OFF_BQ = 256
OFF_BK = 258
OFF_BV = 260
CSTW = 516

_NC = None
LAST_RESULTS = None


def _build_nc():
    import concourse.mybir as mybir
    import concourse.tile as tile
    from concourse import bacc
    from contextlib import ExitStack

    dt = mybir.dt
    f32 = dt.float32
    f16 = dt.float16
    ALU = mybir.AluOpType
    ACTF = mybir.ActivationFunctionType

    nc = bacc.Bacc(
        "TRN2",
        target_bir_lowering=False,
        debug=False,
        enable_asserts=False,
        num_devices=8,
    )

    # host-packed layouts: contiguous per-partition lines per transfer
    xq = nc.dram_tensor("xq", [P, 4, CC, 512], f16, kind="ExternalInput").ap()
    wq2 = nc.dram_tensor("wq2", [P, 2, CC, P], f16, kind="ExternalInput").ap()
    wk2 = nc.dram_tensor("wk2", [P, 2, CC, P], f16, kind="ExternalInput").ap()
    wv2 = nc.dram_tensor("wv2", [P, CC, HD], f16, kind="ExternalInput").ap()
    wp2 = nc.dram_tensor("wp2", [P, 2, C], f16, kind="ExternalInput").ap()
    cst = nc.dram_tensor("cst", [P, CSTW], f16, kind="ExternalInput").ap()
    out = nc.dram_tensor("out", [T, C], f16, kind="ExternalOutput").ap()

    with tile.TileContext(nc) as tc, ExitStack() as ctx:
        persist = ctx.enter_context(tc.tile_pool(name="persist", bufs=1))
        qT_sb = persist.tile([P, 2, T], f16, name="qT")    # [d%128, dchunk, t]
        kT_sb = persist.tile([P, 2, T], f16, name="kT")
        v_sb = persist.tile([P, TC, 2, 2, P], f16, name="v")  # [t%128, tchunk, hpair, hi, 128-padded d]
        yT_sb = persist.tile([P, 2, T], f16, name="yT")
        wp_sb = persist.tile([P, 2, C], f16, name="wps")
        cst_sb = persist.tile([P, CSTW], f16, name="csts")
        dsb = persist.tile([P, 512], f16, name="dsb")
        xT_sb = persist.tile([P, CC, T], f16, name="xTs")
        wq_sb = persist.tile([P, CC, HD], f16, name="wqs")
        wk_sb = persist.tile([P, CC, HD], f16, name="wks")
        wv_sb = persist.tile([P, CC, HD], f16, name="wvs")

        tri_v = cst_sb[:, OFF_TRI:OFF_TRI + P]
        sel_v = cst_sb[:, OFF_SEL:OFF_SEL + P]
        bq_v = cst_sb[:, OFF_BQ:OFF_BQ + 2]
        bk_v = cst_sb[:, OFF_BK:OFF_BK + 2]
        bv_v = cst_sb[:, OFF_BV:OFF_BV + HD].rearrange(
            "p (hp hi d) -> p hp hi d", hi=2, d=D
        )

        ph2 = ctx.enter_context(tc.tile_pool(name="ph2", bufs=3))
        ph3 = ctx.enter_context(tc.tile_pool(name="ph3", bufs=3))
        ps2s = ctx.enter_context(tc.tile_pool(name="ps2s", bufs=2, space="PSUM"))
        ps2a = ctx.enter_context(tc.tile_pool(name="ps2a", bufs=2, space="PSUM"))

        # one-time inits (no inputs needed)
        nc.vector.memset(dsb[:, :], 0.0)
        # constant regions of the padded v operand: zeros + the ones column
        # that makes each AV matmul emit the softmax denominator row
        nc.vector.memset(v_sb[:, :, :, 0, D + 1:P], 0.0)
        nc.vector.memset(v_sb[:, :, :, 1, 1:D], 0.0)
        nc.vector.memset(v_sb[:, :, :, 0, D:D + 1], 1.0)
        nc.vector.memset(v_sb[:, :, :, 1, 0:1], 1.0)

        # ---------------- DMA, first-needed-first ----------------
        # first x quarter rides 4 trigger queues so the first matmul group
        # (which contracts over all 8 cc chunks) unblocks as early as possible
        nc.sync.dma_start(wq_sb[:, :, 0:P], wq2[:, 0, :, :])
        nc.gpsimd.dma_start(wk_sb[:, :, 0:P], wk2[:, 0, :, :])
        nc.sync.dma_start(xT_sb[:, 0:3, 0:512], xq[:, 0, 0:3, :])
        nc.gpsimd.dma_start(xT_sb[:, 3:6, 0:512], xq[:, 0, 3:6, :])
        nc.scalar.dma_start(xT_sb[:, 6:8, 0:512], xq[:, 0, 6:8, :])
        nc.sync.dma_start(wq_sb[:, :, P:HD], wq2[:, 1, :, :])
        nc.gpsimd.dma_start(wk_sb[:, :, P:HD], wk2[:, 1, :, :])
        nc.sync.dma_start(cst_sb[:], cst)
        nc.gpsimd.dma_start(wv_sb[:], wv2)
        for tq in range(1, 4):
            s = slice(tq * 512, (tq + 1) * 512)
            nc.sync.dma_start(xT_sb[:, 0:4, s], xq[:, tq, 0:4, :])
            nc.gpsimd.dma_start(xT_sb[:, 4:8, s], xq[:, tq, 4:8, :])
        nc.gpsimd.dma_start(wp_sb[:], wp2)

        # ---------------- phase 1 units ----------------
        def emit_qk(W_s, bco, dest, co, tsl):
            ps = ps2s.tile([P, 2, 512], f32, tag="s")
            for cc in range(CC):
                nc.tensor.matmul(
                    ps[:, 0, :],
                    W_s[:, cc, co * P:(co + 1) * P],
                    xT_sb[:, cc, tsl * 512:(tsl + 1) * 512],
                    start=(cc == 0),
                    stop=(cc == CC - 1),
                )
            nc.vector.tensor_tensor(
                dest[:, co, tsl * 512:(tsl + 1) * 512],
                ps[:, 0, :],
                bco.to_broadcast([P, 512]),
                ALU.add,
            )

        def emit_v(tj):
            ps = ps2s.tile([P, 2, 512], f32, tag="s")
            for cc in range(CC):
                nc.tensor.matmul(
                    ps[:, 0, 0:HD],
                    xT_sb[:, cc, tj * P:(tj + 1) * P],
                    wv_sb[:, cc, :],
                    start=(cc == 0),
                    stop=(cc == CC - 1),
                )
            psv = ps[:, 0, 0:HD].rearrange("p (hp hi d) -> p hp hi d", hi=2, d=D)
            nc.vector.tensor_tensor(
                v_sb[:, tj, :, 0, 0:D], psv[:, :, 0, :], bv_v[:, :, 0, :], ALU.add
            )
            nc.vector.tensor_tensor(
                v_sb[:, tj, :, 1, D:P], psv[:, :, 1, :], bv_v[:, :, 1, :], ALU.add
            )

        # main phase 1: t-quarters 0-2 of qT/kT + v chunks 0-11. The rest
        # (quarter 3 + v 12-15) becomes attention-phase PE filler. co-outer
        # order so the second group (wk co=0, own DMA queue) never waits on
        # the wq co=1 transfer.
        for tsl in range(3):
            for co in range(2):
                for W_s, boff, dest in (
                    (wq_sb, OFF_BQ, qT_sb),
                    (wk_sb, OFF_BK, kT_sb),
                ):
                    emit_qk(
                        W_s, cst_sb[:, boff + co:boff + co + 1], dest, co, tsl
                    )
            for tj in range(4 * tsl, 4 * tsl + 4):
                emit_v(tj)

        ph1_fill = []
        for co in range(2):
            ph1_fill.append(
                lambda co=co: emit_qk(
                    wk_sb, cst_sb[:, OFF_BK + co:OFF_BK + co + 1], kT_sb, co, 3
                )
            )
            ph1_fill.append(
                lambda co=co: emit_qk(
                    wq_sb, cst_sb[:, OFF_BQ + co:OFF_BQ + co + 1], qT_sb, co, 3
                )
            )
        for tj in range(12, 16):
            ph1_fill.append(lambda tj=tj: emit_v(tj))
        proj_fill = []

        # ---------------- phase 2: attention + woven proj ----------------
        def emit_proj(tj):
            ot = ph3.tile([P, C], f16, tag="ot")
            pps = ps2s.tile([P, 2, 512], f32, tag="s")
            for co in range(2):
                for dc in range(2):
                    nc.tensor.matmul(
                        pps[:, co, :],
                        yT_sb[:, dc, tj * P:(tj + 1) * P],
                        wp_sb[:, dc, co * 512:(co + 1) * 512],
                        start=(dc == 0),
                        stop=(dc == 1),
                    )
                # last i-chunk: ACT has no exp left, split the copies so the
                # tail drains twice as fast
                if tj >= 12 and co == 1:
                    nc.scalar.copy(ot[:, co * 512:(co + 1) * 512], pps[:, co, :])
                else:
                    nc.vector.tensor_copy(
                        ot[:, co * 512:(co + 1) * 512], pps[:, co, :]
                    )
            nc.sync.dma_start(out[tj * P:(tj + 1) * P, :], ot[:])

        def fill(n):
            for _ in range(n):
                if ph1_fill:
                    ph1_fill.pop(0)()
                elif proj_fill:
                    proj_fill.pop(0)()
                else:
                    return

        def emit_norm(hp, i0, av0, av1):
            # den rows were already copied into dsb right after the block's
            # last AV; broadcast + reciprocal + scale into yT
            bps = ps2s.tile([P, 2, 512], f32, tag="s")
            nc.tensor.matmul(
                bps[:, 0, :], sel_v, dsb[:],
                start=True, stop=True, skip_group_check=True,
            )
            rec = ph2.tile([P, 512], f32, tag="rec")
            nc.vector.reciprocal_approx_fast(rec[:, :], bps[:, 0, :])
            nc.vector.tensor_tensor(
                yT_sb[0:D, hp, i0:i0 + 512], av0[0:D, :], rec[0:D, :], ALU.mult
            )
            nc.vector.tensor_tensor(
                yT_sb[D:P, hp, i0:i0 + 512], av1[D:P, :], rec[D:P, :], ALU.mult
            )
            if hp == 1:
                ci = i0 // 512
                for tj in range(4 * ci, 4 * ci + 4):
                    proj_fill.append(lambda tj=tj: emit_proj(tj))

        pending = None
        for ci in range(IC):
            i0 = ci * 512
            njc = 4 * (ci + 1)
            if ci == 3:
                # phase-1 stragglers must land before the blocks that read them
                while ph1_fill:
                    ph1_fill.pop(0)()
            for hp in range(2):
                av0 = ps2a.tile([P, 512], f32, tag="av0")
                av1 = ps2a.tile([P, 512], f32, tag="av1")

                def emit_s(jc):
                    diag = jc >= 4 * ci
                    o = (jc - 4 * ci) if diag else 0
                    c0 = o * P
                    sps = ps2s.tile([P, 2, 512], f32, tag="s")
                    for hi in range(2):
                        bp = D * hi
                        nc.tensor.matmul(
                            sps[:, hi, c0:512],
                            kT_sb[bp:bp + D, hp, jc * P:(jc + 1) * P],
                            qT_sb[bp:bp + D, hp, i0 + c0:i0 + 512],
                            start=True,
                            stop=True,
                            skip_group_check=True,
                        )
                    ex = ph2.tile([P, 2, 512], f16, tag="ex")
                    nc.scalar.activation(
                        ex[:, :, c0:512],
                        sps[:, :, c0:512],
                        ACTF.Exp,
                        scale=float(D) ** -0.5,
                    )
                    if diag:
                        nc.vector.tensor_tensor(
                            ex[:, :, c0:c0 + P],
                            ex[:, :, c0:c0 + P],
                            tri_v[:, None, :].to_broadcast([P, 2, P]),
                            ALU.mult,
                        )
                    return ex, c0

                def emit_av(jc, ex, c0):
                    for hi, av in ((0, av0), (1, av1)):
                        nc.tensor.matmul(
                            av[:, c0:512],
                            v_sb[:, jc, hp, hi, :],
                            ex[:, hi, c0:512],
                            start=(jc == 0),
                            stop=(jc == njc - 1),
                            skip_group_check=True,
                        )

                # score jc+1 issues before AV jc so the in-order PE stream
                # never waits on the ACT exp; fillers pad the PE to ACT rate;
                # the previous block's normalization lands at jc==3, behind a
                # filler, so its broadcast-matmul never waits on the den rows
                pend_av = None
                for jc in range(njc):
                    ex, c0 = emit_s(jc)
                    if pend_av is not None:
                        emit_av(*pend_av)
                    pend_av = (jc, ex, c0)
                    if jc % 4 == 3:
                        fill(1)
                    if jc == 3 and pending is not None:
                        emit_norm(*pending)
                        pending = None
                    if jc == njc - 1 and njc >= 12:
                        fill(1)
                emit_av(*pend_av)
                # den rows -> staging now (on ACT: Copy lives in every table
                # set, and this keeps the latency-critical DVE queue short);
                # the rest of the normalization is emitted in the next block
                nc.scalar.copy(dsb[D:D + 1, :], av0[D:D + 1, :])
                nc.scalar.copy(dsb[0:1, :], av1[0:1, :])
                pending = (hp, i0, av0, av1)
        fill(1)
        emit_norm(*pending)
        while proj_fill:
            proj_fill.pop(0)()
    nc.compile()
    return nc


def _get_nc():
    global _NC
    if _NC is None:
        _NC = _build_nc()
    return _NC


def _pack_inputs(x_b, W_qkv, b_qkv, W_proj, g):
    """Host-side packing for core (batch, head-group g): fp16, DMA-friendly."""
    f16 = np.float16
    s0 = HD * g
    xt = np.ascontiguousarray(x_b.T).astype(f16)          # [C, T]
    xqa = np.ascontiguousarray(
        xt.reshape(CC, P, 4, 512).transpose(1, 2, 0, 3)   # [p, quarter, o, t]
    )

    def wpack(col0):
        w = W_qkv[:, col0:col0 + HD].astype(f16)          # [C, HD]
        return np.ascontiguousarray(w.reshape(CC, P, 2, P).transpose(1, 2, 0, 3))

    wv_ = W_qkv[:, 2 * C + s0:2 * C + s0 + HD].astype(f16)
    wv_p = np.ascontiguousarray(wv_.reshape(CC, P, HD).transpose(1, 0, 2))
    wp_ = W_proj[s0:s0 + HD, :].astype(f16)               # [HD, C]
    wp_p = np.ascontiguousarray(wp_.reshape(2, P, C).transpose(1, 0, 2))

    cstm = np.zeros((P, CSTW), dtype=f16)
    cstm[:, OFF_TRI:OFF_TRI + P] = np.triu(np.ones((P, P), dtype=f16))
    cstm[D, OFF_SEL:OFF_SEL + D] = 1.0
    cstm[0, OFF_SEL + D:OFF_SEL + P] = 1.0
    cstm[:, OFF_BQ:OFF_BQ + 2] = b_qkv[s0:s0 + HD].reshape(2, P).T
    cstm[:, OFF_BK:OFF_BK + 2] = (
        b_qkv[C + s0:C + s0 + HD].reshape(2, P).T
    )
    cstm[:, OFF_BV:OFF_BV + HD] = b_qkv[2 * C + s0:2 * C + s0 + HD]

    return {
        "xq": xqa,
        "wq2": wpack(s0),
        "wk2": wpack(C + s0),
        "wv2": wv_p,
        "wp2": wp_p,
        "cst": np.ascontiguousarray(cstm),
    }


def kernel(x, W_qkv, b_qkv, W_proj, b_proj):
    global LAST_RESULTS
    from concourse import bass_utils

    x = np.asarray(x, dtype=np.float32)
    W_qkv = np.asarray(W_qkv, dtype=np.float32)
    b_qkv = np.asarray(b_qkv, dtype=np.float32)
    W_proj = np.asarray(W_proj, dtype=np.float32)
    b_proj = np.asarray(b_proj, dtype=np.float32)

    nc = _get_nc()
    in_maps = []
    for c in range(8):
        b, g = divmod(c, 4)
        in_maps.append(_pack_inputs(x[b], W_qkv, b_qkv, W_proj, g))

    res = bass_utils.run_bass_kernel_spmd(nc, in_maps, core_ids=list(range(8)))
    LAST_RESULTS = res
    ys = []
    for b in range(2):
        y = res.results[4 * b]["out"].astype(np.float64)
        for g in range(1, 4):
            y = y + res.results[4 * b + g]["out"]
        ys.append((y + b_proj).astype(np.float32))
    return np.stack(ys, axis=0)


# revision 10
# speedup vs baseline: 1.6906x; 1.0509x over previous
"""Causal self-attention (B=2, T=2048, C=1024, H=16) on 8 Trainium2 cores.

Sharding: data-parallel over batch (2) x tensor-parallel over heads (4 groups
of 4 heads). Core c handles batch b = c//4, head group g = c%4 (heads 4g..4g+3).
Each core computes its qkv column slice, full causal TxT attention for its 4
heads, and a partial row-parallel projection. Host sums the 4 partial proj
outputs per batch and adds b_proj.

Device kernel layout notes (v3):
- all matmul operands are fp16: on TRN2 fp32r runs fp32_mode=HIGH (2 PE
  passes -> 2 cycles/row, doubled LDWEIGHTS, and the row-bank conflict
  serializes the two K=64 head-quadrant score matmuls). fp16 is 1 cycle/row,
  enables FWL weight loads, and the hi=0/hi=1 score matmuls (stationary rows
  0-63 / 64-127) genuinely overlap (measured dstart ~4ns). PSUM stays fp32.
- feature-major ("transposed") layouts throughout: qT/kT [d, t] so PE
  contraction dims line up with no on-device transposes
- host pre-packs every DRAM tensor so each DMA moves 2-8KB contiguous
  per-partition lines (small strided lines measured ~85GB/s/queue; packed
  ~200+GB/s), and the first-needed tensors (wq col group 0, x t-quarter 0)
  are triggered first; all small constants ship as one [128, 516] f16 blob
- softmax without max-subtraction (logits ~N(0,1), exp fits fp16); exp runs
  on the ACT engine - the second binding resource (~70us) after the PE
  (~100us); the AV stationary operand embeds an all-ones column so each AV
  matmul also emits the softmax denominator row for free
- denominator rows are partition-broadcast with a single K=128 matmul against
  a constant selector matrix (col j<64 reads row 64 = den0, col j>=64 reads
  row 0 = den1) over a pre-zeroed staging row-pair; reciprocal via the
  custom-DVE reciprocal_approx_fast (~5x faster than exact reciprocal)
- causal masking: upper-triangle j-chunks skipped; diagonal chunks narrow the
  score/exp/AV column range to [o*128, 512) and one [128,128] triangular
  multiplicative mask handles the partial strip
- scheduling: the score matmul for chunk jc+1 issues before the AV for jc so
  the in-order PE stream never waits on ACT; "filler" PE work (projection of
  finished i-chunks + deferred phase-1 work: the last t-quarter of qT/kT and
  v chunks 12-15) is woven in at a fixed cadence to keep the PE at the ACT
  rate. Dense PE occupancy also parks the HAM clock gate at 2.4 GHz (any
  ~3.4us idle window re-throttles the PE to 1.2 GHz).
- normalization lags one block: den-row copies (DVE) issue right after the
  block's last AV, the broadcast-matmul + reciprocal + yT scale are emitted
  inside the NEXT block so the PE never stalls on DVE latency
- PSUM budget (8 banks): "s" tag [128,2,512] bufs=2 (scores, proj, den
  broadcast, and phase-1 qkv groups all share it) + av0/av1 bufs=2 each
- psum->sbuf copies for the projection ride the GpSimd engine (DVE is the
  #3 resource); output tiles stream out per-tile as fp16 on the sync queue
"""

import os
import sys

sys.path.insert(0, "/opt/trn_rl_repo")

import numpy as np

P = 128
T = 2048
C = 1024
D = 64
HPC = 4          # heads per core
HD = HPC * D     # 256 qkv columns per core
CC = C // P      # 8 contraction chunks
TC = T // P      # 16 t-chunks of 128
IC = T // 512    # 4 i-chunks of 512

# const blob column offsets
OFF_TRI = 0
OFF_SEL = 128
OFF_BQ = 256
OFF_BK = 258
OFF_BV = 260
CSTW = 516

_NC = None
LAST_RESULTS = None


def _build_nc():
    import concourse.mybir as mybir
    import concourse.tile as tile
    from concourse import bacc
    from contextlib import ExitStack

    dt = mybir.dt
    f32 = dt.float32
    f16 = dt.float16
    ALU = mybir.AluOpType
    ACTF = mybir.ActivationFunctionType

    nc = bacc.Bacc(
        "TRN2",
        target_bir_lowering=False,
        debug=False,
        enable_asserts=False,
        num_devices=8,
    )

    # host-packed layouts: contiguous per-partition lines per transfer
    xq = nc.dram_tensor("xq", [P, 4, CC, 512], f16, kind="ExternalInput").ap()
    wq2 = nc.dram_tensor("wq2", [P, 2, CC, P], f16, kind="ExternalInput").ap()
    wk2 = nc.dram_tensor("wk2", [P, 2, CC, P], f16, kind="ExternalInput").ap()
    wv2 = nc.dram_tensor("wv2", [P, CC, HD], f16, kind="ExternalInput").ap()
    wp2 = nc.dram_tensor("wp2", [P, 2, C], f16, kind="ExternalInput").ap()
    cst = nc.dram_tensor("cst", [P, CSTW], f16, kind="ExternalInput").ap()
    out = nc.dram_tensor("out", [T, C], f16, kind="ExternalOutput").ap()

    with tile.TileContext(nc) as tc, ExitStack() as ctx:
        persist = ctx.enter_context(tc.tile_pool(name="persist", bufs=1))
        qT_sb = persist.tile([P, 2, T], f16, name="qT")    # [d%128, dchunk, t]
        kT_sb = persist.tile([P, 2, T], f16, name="kT")
        v_sb = persist.tile([P, TC, 2, 2, P], f16, name="v")  # [t%128, tchunk, hpair, hi, 128-padded d]
        yT_sb = persist.tile([P, 2, T], f16, name="yT")
        wp_sb = persist.tile([P, 2, C], f16, name="wps")
        cst_sb = persist.tile([P, CSTW], f16, name="csts")
        dsb = persist.tile([P, 512], f16, name="dsb")
        # x is t-quarter-major and the qk weights co-major so every DMA
        # lands with 2-8KB contiguous runs on BOTH sides (small runs
        # measured ~8 B/ns per DMA engine; 4KB runs ~21 B/ns)
        xs_sb = persist.tile([P, 4, CC, 512], f16, name="xss")
        wq_sb = persist.tile([P, 2, CC, P], f16, name="wqs")
        wk_sb = persist.tile([P, 2, CC, P], f16, name="wks")
        wv_sb = persist.tile([P, CC, HD], f16, name="wvs")

        tri_v = cst_sb[:, OFF_TRI:OFF_TRI + P]
        sel_v = cst_sb[:, OFF_SEL:OFF_SEL + P]
        bq_v = cst_sb[:, OFF_BQ:OFF_BQ + 2]
        bk_v = cst_sb[:, OFF_BK:OFF_BK + 2]
        bv_v = cst_sb[:, OFF_BV:OFF_BV + HD].rearrange(
            "p (hp hi d) -> p hp hi d", hi=2, d=D
        )

        ph2 = ctx.enter_context(tc.tile_pool(name="ph2", bufs=3))
        ph3 = ctx.enter_context(tc.tile_pool(name="ph3", bufs=3))
        ps2s = ctx.enter_context(tc.tile_pool(name="ps2s", bufs=2, space="PSUM"))
        ps2a = ctx.enter_context(tc.tile_pool(name="ps2a", bufs=2, space="PSUM"))

        # one-time inits (no inputs needed)
        nc.vector.memset(dsb[:, :], 0.0)
        # constant regions of the padded v operand: zeros + the ones column
        # that makes each AV matmul emit the softmax denominator row
        nc.vector.memset(v_sb[:, :, :, 0, D + 1:P], 0.0)
        nc.vector.memset(v_sb[:, :, :, 1, 1:D], 0.0)
        nc.vector.memset(v_sb[:, :, :, 0, D:D + 1], 1.0)
        nc.vector.memset(v_sb[:, :, :, 1, 0:1], 1.0)

        # ---------------- DMA, first-needed-first ----------------
        # first x quarter rides 4 trigger queues so the first matmul group
        # (which contracts over all 8 cc chunks) unblocks as early as possible
        nc.sync.dma_start(wq_sb[:, 0, :, :], wq2[:, 0, :, :])
        nc.gpsimd.dma_start(wk_sb[:, 0, :, :], wk2[:, 0, :, :])
        nc.sync.dma_start(xs_sb[:, 0, 0:3, :], xq[:, 0, 0:3, :])
        nc.gpsimd.dma_start(xs_sb[:, 0, 3:6, :], xq[:, 0, 3:6, :])
        nc.scalar.dma_start(xs_sb[:, 0, 6:8, :], xq[:, 0, 6:8, :])
        nc.sync.dma_start(wq_sb[:, 1, :, :], wq2[:, 1, :, :])
        nc.gpsimd.dma_start(wk_sb[:, 1, :, :], wk2[:, 1, :, :])
        nc.sync.dma_start(cst_sb[:], cst)
        nc.gpsimd.dma_start(wv_sb[:], wv2)
        for tq in range(1, 4):
            nc.sync.dma_start(xs_sb[:, tq, 0:4, :], xq[:, tq, 0:4, :])
            nc.gpsimd.dma_start(xs_sb[:, tq, 4:8, :], xq[:, tq, 4:8, :])
        nc.gpsimd.dma_start(wp_sb[:], wp2)

        # ---------------- phase 1 units ----------------
        def emit_qk(W_s, bco, dest, co, tsl):
            ps = ps2s.tile([P, 2, 512], f32, tag="s")
            for cc in range(CC):
                nc.tensor.matmul(
                    ps[:, 0, :],
                    W_s[:, co, cc, :],
                    xs_sb[:, tsl, cc, :],
                    start=(cc == 0),
                    stop=(cc == CC - 1),
                )
            nc.vector.tensor_tensor(
                dest[:, co, tsl * 512:(tsl + 1) * 512],
                ps[:, 0, :],
                bco.to_broadcast([P, 512]),
                ALU.add,
            )

        def emit_v(tj):
            ps = ps2s.tile([P, 2, 512], f32, tag="s")
            for cc in range(CC):
                nc.tensor.matmul(
                    ps[:, 0, 0:HD],
                    xs_sb[:, tj // 4, cc, (tj % 4) * P:(tj % 4 + 1) * P],
                    wv_sb[:, cc, :],
                    start=(cc == 0),
                    stop=(cc == CC - 1),
                )
            psv = ps[:, 0, 0:HD].rearrange("p (hp hi d) -> p hp hi d", hi=2, d=D)
            nc.vector.tensor_tensor(
                v_sb[:, tj, :, 0, 0:D], psv[:, :, 0, :], bv_v[:, :, 0, :], ALU.add
            )
            nc.vector.tensor_tensor(
                v_sb[:, tj, :, 1, D:P], psv[:, :, 1, :], bv_v[:, :, 1, :], ALU.add
            )

        # main phase 1: t-quarters 0-2 of qT/kT + v chunks 0-11. The rest
        # (quarter 3 + v 12-15) becomes attention-phase PE filler. co-outer
        # order so the second group (wk co=0, own DMA queue) never waits on
        # the wq co=1 transfer.
        for tsl in range(2):
            for co in range(2):
                for W_s, boff, dest in (
                    (wq_sb, OFF_BQ, qT_sb),
                    (wk_sb, OFF_BK, kT_sb),
                ):
                    emit_qk(
                        W_s, cst_sb[:, boff + co:boff + co + 1], dest, co, tsl
                    )
            for tj in range(4 * tsl, 4 * tsl + 4):
                emit_v(tj)

        # quarters 2-3 of phase 1 are filler inside the attention phase (the
        # PE runs ~640ns/chunk vs ACT's ~1.1us/chunk there); tagged with the
        # t-quarter so blocks that need them can force-drain first
        ph1_fill = []
        for tsl in (2, 3):
            for co in range(2):
                ph1_fill.append((tsl, lambda co=co, tsl=tsl: emit_qk(
                    wk_sb, cst_sb[:, OFF_BK + co:OFF_BK + co + 1], kT_sb, co, tsl
                )))
                ph1_fill.append((tsl, lambda co=co, tsl=tsl: emit_qk(
                    wq_sb, cst_sb[:, OFF_BQ + co:OFF_BQ + co + 1], qT_sb, co, tsl
                )))
            for tj in range(4 * tsl, 4 * tsl + 4):
                ph1_fill.append((tsl, lambda tj=tj: emit_v(tj)))
        proj_fill = []

        # ---------------- phase 2: attention + woven proj ----------------
        def emit_proj(tj):
            ot = ph3.tile([P, C], f16, tag="ot")
            pps = ps2s.tile([P, 2, 512], f32, tag="s")
            for co in range(2):
                for dc in range(2):
                    nc.tensor.matmul(
                        pps[:, co, :],
                        yT_sb[:, dc, tj * P:(tj + 1) * P],
                        wp_sb[:, dc, co * 512:(co + 1) * 512],
                        start=(dc == 0),
                        stop=(dc == 1),
                    )
                # last i-chunk: ACT has no exp left, split the copies so the
                # tail drains twice as fast
                if tj >= 12 and co == 1:
                    nc.scalar.copy(ot[:, co * 512:(co + 1) * 512], pps[:, co, :])
                else:
                    nc.vector.tensor_copy(
                        ot[:, co * 512:(co + 1) * 512], pps[:, co, :]
                    )
            nc.sync.dma_start(out[tj * P:(tj + 1) * P, :], ot[:])

        def fill(n):
            for _ in range(n):
                if ph1_fill:
                    ph1_fill.pop(0)[1]()
                elif proj_fill:
                    proj_fill.pop(0)()
                else:
                    return

        def drain_ph1(upto_quarter):
            while ph1_fill and ph1_fill[0][0] <= upto_quarter:
                ph1_fill.pop(0)[1]()

        def emit_norm(hp, i0, av0, av1):
            # den rows were already copied into dsb right after the block's
            # last AV; broadcast + reciprocal + scale into yT
            bps = ps2s.tile([P, 2, 512], f32, tag="s")
            nc.tensor.matmul(
                bps[:, 0, :], sel_v, dsb[:],
                start=True, stop=True, skip_group_check=True,
            )
            rec = ph2.tile([P, 512], f32, tag="rec")
            nc.vector.reciprocal_approx_fast(rec[:, :], bps[:, 0, :])
            nc.vector.tensor_tensor(
                yT_sb[0:D, hp, i0:i0 + 512], av0[0:D, :], rec[0:D, :], ALU.mult
            )
            nc.vector.tensor_tensor(
                yT_sb[D:P, hp, i0:i0 + 512], av1[D:P, :], rec[D:P, :], ALU.mult
            )
            if hp == 1:
                ci = i0 // 512
                for tj in range(4 * ci, 4 * ci + 4):
                    proj_fill.append(lambda tj=tj: emit_proj(tj))

        pending = None
        FILL_QUOTA = {4: 3, 8: 5, 12: 7, 16: 8}
        for ci in range(IC):
            i0 = ci * 512
            njc = 4 * (ci + 1)
            if ci >= 2:
                # phase-1 stragglers must land before the blocks that read them
                drain_ph1(ci + 1)
            for hp in range(2):
                av0 = ps2a.tile([P, 512], f32, tag="av0")
                av1 = ps2a.tile([P, 512], f32, tag="av1")

                def emit_s(jc):
                    diag = jc >= 4 * ci
                    o = (jc - 4 * ci) if diag else 0
                    c0 = o * P
                    sps = ps2s.tile([P, 2, 512], f32, tag="s")
                    for hi in range(2):
                        bp = D * hi
                        nc.tensor.matmul(
                            sps[:, hi, c0:512],
                            kT_sb[bp:bp + D, hp, jc * P:(jc + 1) * P],
                            qT_sb[bp:bp + D, hp, i0 + c0:i0 + 512],
                            start=True,
                            stop=True,
                            skip_group_check=True,
                        )
                    ex = ph2.tile([P, 2, 512], f16, tag="ex")
                    nc.scalar.activation(
                        ex[:, :, c0:512],
                        sps[:, :, c0:512],
                        ACTF.Exp,
                        scale=float(D) ** -0.5,
                    )
                    if diag:
                        nc.vector.tensor_tensor(
                            ex[:, :, c0:c0 + P],
                            ex[:, :, c0:c0 + P],
                            tri_v[:, None, :].to_broadcast([P, 2, P]),
                            ALU.mult,
                        )
                    return ex, c0

                def emit_av(jc, ex, c0):
                    for hi, av in ((0, av0), (1, av1)):
                        nc.tensor.matmul(
                            av[:, c0:512],
                            v_sb[:, jc, hp, hi, :],
                            ex[:, hi, c0:512],
                            start=(jc == 0),
                            stop=(jc == njc - 1),
                            skip_group_check=True,
                        )

                # score jc+1 issues before AV jc so the in-order PE stream
                # never waits on the ACT exp; fillers pad the PE to ACT rate;
                # the previous block's normalization lands at jc==3, behind a
                # filler, so its broadcast-matmul never waits on the den rows
                pend_av = None
                quota = FILL_QUOTA[njc]
                for jc in range(njc):
                    ex, c0 = emit_s(jc)
                    if pend_av is not None:
                        emit_av(*pend_av)
                    pend_av = (jc, ex, c0)
                    if jc >= 1 and quota > 0:
                        fill(1)
                        quota -= 1
                    if jc == 3 and pending is not None:
                        emit_norm(*pending)
                        pending = None
                emit_av(*pend_av)
                # den rows -> staging now; the rest of the normalization is
                # emitted inside the next block so the PE never stalls on it
                nc.vector.tensor_copy(dsb[D:D + 1, :], av0[D:D + 1, :])
                nc.vector.tensor_copy(dsb[0:1, :], av1[0:1, :])
                pending = (hp, i0, av0, av1)
        fill(1)
        emit_norm(*pending)
        while proj_fill:
            proj_fill.pop(0)()
    nc.compile()
    return nc


def _get_nc():
    global _NC
    if _NC is None:
        _NC = _build_nc()
    return _NC


def _pack_inputs(x_b, W_qkv, b_qkv, W_proj, g):
    """Host-side packing for core (batch, head-group g): fp16, DMA-friendly."""
    f16 = np.float16
    s0 = HD * g
    xt = np.ascontiguousarray(x_b.T).astype(f16)          # [C, T]
    xqa = np.ascontiguousarray(
        xt.reshape(CC, P, 4, 512).transpose(1, 2, 0, 3)   # [p, quarter, o, t]
    )

    def wpack(col0):
        w = W_qkv[:, col0:col0 + HD].astype(f16)          # [C, HD]
        return np.ascontiguousarray(w.reshape(CC, P, 2, P).transpose(1, 2, 0, 3))

    wv_ = W_qkv[:, 2 * C + s0:2 * C + s0 + HD].astype(f16)
    wv_p = np.ascontiguousarray(wv_.reshape(CC, P, HD).transpose(1, 0, 2))
    wp_ = W_proj[s0:s0 + HD, :].astype(f16)               # [HD, C]
    wp_p = np.ascontiguousarray(wp_.reshape(2, P, C).transpose(1, 0, 2))

    cstm = np.zeros((P, CSTW), dtype=f16)
    cstm[:, OFF_TRI:OFF_TRI + P] = np.triu(np.ones((P, P), dtype=f16))
    cstm[D, OFF_SEL:OFF_SEL + D] = 1.0
    cstm[0, OFF_SEL + D:OFF_SEL + P] = 1.0
    cstm[:, OFF_BQ:OFF_BQ + 2] = b_qkv[s0:s0 + HD].reshape(2, P).T
    cstm[:, OFF_BK:OFF_BK + 2] = (
        b_qkv[C + s0:C + s0 + HD].reshape(2, P).T
    )
    cstm[:, OFF_BV:OFF_BV + HD] = b_qkv[2 * C + s0:2 * C + s0 + HD]

    return {
        "xq": xqa,
        "wq2": wpack(s0),
        "wk2": wpack(C + s0),
        "wv2": wv_p,
        "wp2": wp_p,
        "cst": np.ascontiguousarray(cstm),
    }


def kernel(x, W_qkv, b_qkv, W_proj, b_proj):
    global LAST_RESULTS
    from concourse import bass_utils

    x = np.asarray(x, dtype=np.float32)
    W_qkv = np.asarray(W_qkv, dtype=np.float32)
    b_qkv = np.asarray(b_qkv, dtype=np.float32)
    W_proj = np.asarray(W_proj, dtype=np.float32)
    b_proj = np.asarray(b_proj, dtype=np.float32)

    nc = _get_nc()
    in_maps = []
    for c in range(8):
        b, g = divmod(c, 4)
        in_maps.append(_pack_inputs(x[b], W_qkv, b_qkv, W_proj, g))

    res = bass_utils.run_bass_kernel_spmd(nc, in_maps, core_ids=list(range(8)))
    LAST_RESULTS = res
    ys = []
    for b in range(2):
        y = res.results[4 * b]["out"].astype(np.float64)
        for g in range(1, 4):
            y = y + res.results[4 * b + g]["out"]
        ys.append((y + b_proj).astype(np.float32))
    return np.stack(ys, axis=0)


# revision 12
# speedup vs baseline: 1.7204x; 1.0176x over previous
"""Causal self-attention (B=2, T=2048, C=1024, H=16) on 8 Trainium2 cores.

Sharding: data-parallel over batch (2) x tensor-parallel over heads (4 groups
of 4 heads). Core c handles batch b = c//4, head group g = c%4 (heads 4g..4g+3).
Each core computes its qkv column slice, full causal TxT attention for its 4
heads, and a partial row-parallel projection. Host sums the 4 partial proj
outputs per batch and adds b_proj.

Device kernel layout notes (v3):
- all matmul operands are fp16: on TRN2 fp32r runs fp32_mode=HIGH (2 PE
  passes -> 2 cycles/row, doubled LDWEIGHTS, and the row-bank conflict
  serializes the two K=64 head-quadrant score matmuls). fp16 is 1 cycle/row,
  enables FWL weight loads, and the hi=0/hi=1 score matmuls (stationary rows
  0-63 / 64-127) genuinely overlap (measured dstart ~4ns). PSUM stays fp32.
- feature-major ("transposed") layouts throughout: qT/kT [d, t] so PE
  contraction dims line up with no on-device transposes
- host pre-packs every DRAM tensor so each DMA moves 2-8KB contiguous
  per-partition lines (small strided lines measured ~85GB/s/queue; packed
  ~200+GB/s), and the first-needed tensors (wq col group 0, x t-quarter 0)
  are triggered first; all small constants ship as one [128, 516] f16 blob
- softmax without max-subtraction (logits ~N(0,1), exp fits fp16); exp runs
  on the ACT engine - the second binding resource (~70us) after the PE
  (~100us); the AV stationary operand embeds an all-ones column so each AV
  matmul also emits the softmax denominator row for free
- denominator rows are partition-broadcast with a single K=128 matmul against
  a constant selector matrix (col j<64 reads row 64 = den0, col j>=64 reads
  row 0 = den1) over a pre-zeroed staging row-pair; reciprocal via the
  custom-DVE reciprocal_approx_fast (~5x faster than exact reciprocal)
- causal masking: upper-triangle j-chunks skipped; diagonal chunks narrow the
  score/exp/AV column range to [o*128, 512) and one [128,128] triangular
  multiplicative mask handles the partial strip
- scheduling: the score matmul for chunk jc+1 issues before the AV for jc so
  the in-order PE stream never waits on ACT; "filler" PE work (projection of
  finished i-chunks + deferred phase-1 work: the last t-quarter of qT/kT and
  v chunks 12-15) is woven in at a fixed cadence to keep the PE at the ACT
  rate. Dense PE occupancy also parks the HAM clock gate at 2.4 GHz (any
  ~3.4us idle window re-throttles the PE to 1.2 GHz).
- normalization lags one block: den-row copies (DVE) issue right after the
  block's last AV, the broadcast-matmul + reciprocal + yT scale are emitted
  inside the NEXT block so the PE never stalls on DVE latency
- PSUM budget (8 banks): "s" tag [128,2,512] bufs=2 (scores, proj, den
  broadcast, and phase-1 qkv groups all share it) + av0/av1 bufs=2 each
- psum->sbuf copies for the projection ride the GpSimd engine (DVE is the
  #3 resource); output tiles stream out per-tile as fp16 on the sync queue
"""

import os
import sys

sys.path.insert(0, "/opt/trn_rl_repo")

import numpy as np

P = 128
T = 2048
C = 1024
D = 64
HPC = 4          # heads per core
HD = HPC * D     # 256 qkv columns per core
CC = C // P      # 8 contraction chunks
TC = T // P      # 16 t-chunks of 128
IC = T // 512    # 4 i-chunks of 512

# const blob column offsets
OFF_TRI = 0
OFF_SEL = 128
OFF_BQ = 256
OFF_BK = 258
OFF_BV = 260
CSTW = 516

_NC = None
LAST_RESULTS = None


def _build_nc():
    import concourse.mybir as mybir
    import concourse.tile as tile
    from concourse import bacc
    from contextlib import ExitStack

    dt = mybir.dt
    f32 = dt.float32
    f16 = dt.float16
    ALU = mybir.AluOpType
    ACTF = mybir.ActivationFunctionType

    nc = bacc.Bacc(
        "TRN2",
        target_bir_lowering=False,
        debug=False,
        enable_asserts=False,
        num_devices=8,
    )

    # host-packed layouts: contiguous per-partition lines per transfer
    xq = nc.dram_tensor("xq", [P, 4, CC, 512], f16, kind="ExternalInput").ap()
    wq2 = nc.dram_tensor("wq2", [P, 2, CC, P], f16, kind="ExternalInput").ap()
    wk2 = nc.dram_tensor("wk2", [P, 2, CC, P], f16, kind="ExternalInput").ap()
    wv2 = nc.dram_tensor("wv2", [P, CC, HD], f16, kind="ExternalInput").ap()
    wp2 = nc.dram_tensor("wp2", [P, 2, C], f16, kind="ExternalInput").ap()
    cst = nc.dram_tensor("cst", [P, CSTW], f16, kind="ExternalInput").ap()
    out = nc.dram_tensor("out", [T, C], f16, kind="ExternalOutput").ap()

    with tile.TileContext(nc) as tc, ExitStack() as ctx:
        persist = ctx.enter_context(tc.tile_pool(name="persist", bufs=1))
        qT_sb = persist.tile([P, 2, T], f16, name="qT")    # [d%128, dchunk, t]
        kT_sb = persist.tile([P, 2, T], f16, name="kT")
        v_sb = persist.tile([P, TC, 2, 2, P], f16, name="v")  # [t%128, tchunk, hpair, hi, 128-padded d]
        yT_sb = persist.tile([P, 2, T], f16, name="yT")
        wp_sb = persist.tile([P, 2, C], f16, name="wps")
        cst_sb = persist.tile([P, CSTW], f16, name="csts")
        dsb = persist.tile([P, 512], f16, name="dsb")
        # x is t-quarter-major and the qk weights co-major so every DMA
        # lands with 2-8KB contiguous runs on BOTH sides (small runs
        # measured ~8 B/ns per DMA engine; 4KB runs ~21 B/ns)
        xs_sb = persist.tile([P, 4, CC, 512], f16, name="xss")
        wq_sb = persist.tile([P, 2, CC, P], f16, name="wqs")
        wk_sb = persist.tile([P, 2, CC, P], f16, name="wks")
        wv_sb = persist.tile([P, CC, HD], f16, name="wvs")

        tri_v = cst_sb[:, OFF_TRI:OFF_TRI + P]
        sel_v = cst_sb[:, OFF_SEL:OFF_SEL + P]
        bq_v = cst_sb[:, OFF_BQ:OFF_BQ + 2]
        bk_v = cst_sb[:, OFF_BK:OFF_BK + 2]
        bv_v = cst_sb[:, OFF_BV:OFF_BV + HD].rearrange(
            "p (hp hi d) -> p hp hi d", hi=2, d=D
        )

        ph2 = ctx.enter_context(tc.tile_pool(name="ph2", bufs=3))
        ph3 = ctx.enter_context(tc.tile_pool(name="ph3", bufs=3))
        ps2s = ctx.enter_context(tc.tile_pool(name="ps2s", bufs=2, space="PSUM"))
        ps2a = ctx.enter_context(tc.tile_pool(name="ps2a", bufs=2, space="PSUM"))

        # one-time inits (no inputs needed)
        nc.vector.memset(dsb[:, :], 0.0)
        # constant regions of the padded v operand: zeros + the ones column
        # that makes each AV matmul emit the softmax denominator row
        nc.vector.memset(v_sb[:, :, :, 0, D + 1:P], 0.0)
        nc.vector.memset(v_sb[:, :, :, 1, 1:D], 0.0)
        nc.vector.memset(v_sb[:, :, :, 0, D:D + 1], 1.0)
        nc.vector.memset(v_sb[:, :, :, 1, 0:1], 1.0)

        # ---------------- DMA, first-needed-first ----------------
        # first x quarter rides 4 trigger queues so the first matmul group
        # (which contracts over all 8 cc chunks) unblocks as early as possible
        nc.sync.dma_start(xs_sb[:, 0, 0:3, :], xq[:, 0, 0:3, :])
        nc.gpsimd.dma_start(xs_sb[:, 0, 3:7, :], xq[:, 0, 3:7, :])
        nc.scalar.dma_start(wq_sb[:, 0, :, :], wq2[:, 0, :, :])
        nc.scalar.dma_start(xs_sb[:, 0, 7:8, :], xq[:, 0, 7:8, :])
        nc.gpsimd.dma_start(wk_sb[:, 0, :, :], wk2[:, 0, :, :])
        nc.sync.dma_start(cst_sb[:], cst)
        nc.sync.dma_start(wq_sb[:, 1, :, :], wq2[:, 1, :, :])
        nc.gpsimd.dma_start(wk_sb[:, 1, :, :], wk2[:, 1, :, :])
        nc.gpsimd.dma_start(wv_sb[:], wv2)
        for tq in range(1, 4):
            nc.sync.dma_start(xs_sb[:, tq, 0:4, :], xq[:, tq, 0:4, :])
            nc.gpsimd.dma_start(xs_sb[:, tq, 4:8, :], xq[:, tq, 4:8, :])
        nc.gpsimd.dma_start(wp_sb[:], wp2)

        # ---------------- phase 1 units ----------------
        def emit_qk(W_s, bco, dest, co, tsl):
            ps = ps2s.tile([P, 2, 512], f32, tag="s")
            for cc in range(CC):
                nc.tensor.matmul(
                    ps[:, 0, :],
                    W_s[:, co, cc, :],
                    xs_sb[:, tsl, cc, :],
                    start=(cc == 0),
                    stop=(cc == CC - 1),
                )
            nc.vector.tensor_tensor(
                dest[:, co, tsl * 512:(tsl + 1) * 512],
                ps[:, 0, :],
                bco.to_broadcast([P, 512]),
                ALU.add,
            )

        def emit_v(tj):
            ps = ps2s.tile([P, 2, 512], f32, tag="s")
            for cc in range(CC):
                nc.tensor.matmul(
                    ps[:, 0, 0:HD],
                    xs_sb[:, tj // 4, cc, (tj % 4) * P:(tj % 4 + 1) * P],
                    wv_sb[:, cc, :],
                    start=(cc == 0),
                    stop=(cc == CC - 1),
                )
            psv = ps[:, 0, 0:HD].rearrange("p (hp hi d) -> p hp hi d", hi=2, d=D)
            nc.vector.tensor_tensor(
                v_sb[:, tj, :, 0, 0:D], psv[:, :, 0, :], bv_v[:, :, 0, :], ALU.add
            )
            nc.vector.tensor_tensor(
                v_sb[:, tj, :, 1, D:P], psv[:, :, 1, :], bv_v[:, :, 1, :], ALU.add
            )

        # main phase 1: t-quarters 0-2 of qT/kT + v chunks 0-11. The rest
        # (quarter 3 + v 12-15) becomes attention-phase PE filler. co-outer
        # order so the second group (wk co=0, own DMA queue) never waits on
        # the wq co=1 transfer.
        for tsl in range(2):
            for co in range(2):
                for W_s, boff, dest in (
                    (wq_sb, OFF_BQ, qT_sb),
                    (wk_sb, OFF_BK, kT_sb),
                ):
                    emit_qk(
                        W_s, cst_sb[:, boff + co:boff + co + 1], dest, co, tsl
                    )
            for tj in range(4 * tsl, 4 * tsl + 4):
                emit_v(tj)

        # quarters 2-3 of phase 1 are filler inside the attention phase (the
        # PE runs ~640ns/chunk vs ACT's ~1.1us/chunk there); tagged with the
        # t-quarter so blocks that need them can force-drain first
        ph1_fill = []
        for tsl in (2, 3):
            for co in range(2):
                ph1_fill.append((tsl, "qk", lambda co=co, tsl=tsl: emit_qk(
                    wk_sb, cst_sb[:, OFF_BK + co:OFF_BK + co + 1], kT_sb, co, tsl
                )))
                ph1_fill.append((tsl, "qk", lambda co=co, tsl=tsl: emit_qk(
                    wq_sb, cst_sb[:, OFF_BQ + co:OFF_BQ + co + 1], qT_sb, co, tsl
                )))
            for tj in range(4 * tsl, 4 * tsl + 4):
                ph1_fill.append((tsl, "v", lambda tj=tj: emit_v(tj)))
        proj_fill = []

        # ---------------- phase 2: attention + woven proj ----------------
        ot_tiles = {}

        def emit_proj_half(tj, co):
            if co == 0:
                ot_tiles[tj] = ph3.tile([P, C], f16, tag="ot", name=f"ot{tj}")
            ot = ot_tiles[tj]
            pps = ps2s.tile([P, 2, 512], f32, tag="s")
            for dc in range(2):
                nc.tensor.matmul(
                    pps[:, co, :],
                    yT_sb[:, dc, tj * P:(tj + 1) * P],
                    wp_sb[:, dc, co * 512:(co + 1) * 512],
                    start=(dc == 0),
                    stop=(dc == 1),
                )
            # last i-chunk: ACT has no exp left, split the copies so the
            # tail drains twice as fast
            if tj >= 12 and co == 1:
                nc.scalar.copy(ot[:, co * 512:(co + 1) * 512], pps[:, co, :])
            else:
                nc.vector.tensor_copy(
                    ot[:, co * 512:(co + 1) * 512], pps[:, co, :]
                )
            if co == 1:
                nc.sync.dma_start(out[tj * P:(tj + 1) * P, :], ot[:])
                del ot_tiles[tj]

        # emitted-work accounting (ns) to pace fillers: the PE stream should
        # stay at least as long as the ACT (exp) stream it depends on
        clk = {"pe": 0.0, "act": 0.0}

        def fill(n):
            for _ in range(n):
                if ph1_fill:
                    q, kind, fn = ph1_fill.pop(0)
                    fn()
                    clk["pe"] += 1707.0 if kind == "qk" else 853.0
                elif proj_fill:
                    proj_fill.pop(0)()
                    clk["pe"] += 480.0
                else:
                    return

        def fill_to_rate():
            while (ph1_fill or proj_fill) and clk["act"] > clk["pe"]:
                fill(1)

        def drain_ph1(upto_quarter):
            while ph1_fill and ph1_fill[0][0] <= upto_quarter:
                q, kind, fn = ph1_fill.pop(0)
                fn()

        def emit_norm(hp, i0, av0, av1):
            # den rows were already copied into dsb right after the block's
            # last AV; broadcast + reciprocal + scale into yT
            bps = ps2s.tile([P, 2, 512], f32, tag="s")
            nc.tensor.matmul(
                bps[:, 0, :], sel_v, dsb[:],
                start=True, stop=True, skip_group_check=True,
            )
            rec = ph2.tile([P, 512], f32, tag="rec")
            nc.vector.reciprocal_approx_fast(rec[:, :], bps[:, 0, :])
            nc.vector.tensor_tensor(
                yT_sb[0:D, hp, i0:i0 + 512], av0[0:D, :], rec[0:D, :], ALU.mult
            )
            nc.vector.tensor_tensor(
                yT_sb[D:P, hp, i0:i0 + 512], av1[D:P, :], rec[D:P, :], ALU.mult
            )
            if hp == 1:
                ci = i0 // 512
                for tj in range(4 * ci, 4 * ci + 4):
                    for co in range(2):
                        proj_fill.append(
                            lambda tj=tj, co=co: emit_proj_half(tj, co)
                        )

        pending = None
        for ci in range(IC):
            i0 = ci * 512
            njc = 4 * (ci + 1)
            if ci >= 2:
                # phase-1 stragglers must land before the blocks that read
                # them (quarter ci for the kT/qT/v this i-chunk touches)
                drain_ph1(ci)
            for hp in range(2):
                av0 = ps2a.tile([P, 512], f32, tag="av0")
                av1 = ps2a.tile([P, 512], f32, tag="av1")

                def emit_s(jc):
                    diag = jc >= 4 * ci
                    o = (jc - 4 * ci) if diag else 0
                    c0 = o * P
                    sps = ps2s.tile([P, 2, 512], f32, tag="s")
                    for hi in range(2):
                        bp = D * hi
                        nc.tensor.matmul(
                            sps[:, hi, c0:512],
                            kT_sb[bp:bp + D, hp, jc * P:(jc + 1) * P],
                            qT_sb[bp:bp + D, hp, i0 + c0:i0 + 512],
                            start=True,
                            stop=True,
                            skip_group_check=True,
                        )
                    ex = ph2.tile([P, 2, 512], f16, tag="ex")
                    nc.scalar.activation(
                        ex[:, :, c0:512],
                        sps[:, :, c0:512],
                        ACTF.Exp,
                        scale=float(D) ** -0.5,
                    )
                    if diag:
                        nc.vector.tensor_tensor(
                            ex[:, :, c0:c0 + P],
                            ex[:, :, c0:c0 + P],
                            tri_v[:, None, :].to_broadcast([P, 2, P]),
                            ALU.mult,
                        )
                    return ex, c0

                def emit_av(jc, ex, c0):
                    for hi, av in ((0, av0), (1, av1)):
                        nc.tensor.matmul(
                            av[:, c0:512],
                            v_sb[:, jc, hp, hi, :],
                            ex[:, hi, c0:512],
                            start=(jc == 0),
                            stop=(jc == njc - 1),
                            skip_group_check=True,
                        )

                # score jc+1 issues before AV jc so the in-order PE stream
                # never waits on the ACT exp; fillers pad the PE to ACT rate;
                # the previous block's normalization lands at jc==3, behind a
                # filler, so its broadcast-matmul never waits on the den rows
                pend_av = None
                for jc in range(njc):
                    ex, c0 = emit_s(jc)
                    w = 512 - (jc - 4 * ci) * P if jc >= 4 * ci else 512
                    clk["pe"] += w / 2.4 + 8
                    clk["act"] += 2 * w / 1.2 + 160
                    if pend_av is not None:
                        emit_av(*pend_av)
                        clk["pe"] += 2 * (512 - pend_av[2]) / 2.4 + 16
                    pend_av = (jc, ex, c0)
                    if jc >= 1:
                        fill_to_rate()
                    if jc == 3 and pending is not None:
                        emit_norm(*pending)
                        pending = None
                        clk["pe"] += 230.0
                emit_av(*pend_av)
                clk["pe"] += 2 * (512 - pend_av[2]) / 2.4 + 16
                # den rows -> staging now; the rest of the normalization is
                # emitted inside the next block so the PE never stalls on it
                nc.vector.tensor_copy(dsb[D:D + 1, :], av0[D:D + 1, :])
                nc.vector.tensor_copy(dsb[0:1, :], av1[0:1, :])
                pending = (hp, i0, av0, av1)
        fill(1)
        emit_norm(*pending)
        while proj_fill:
            proj_fill.pop(0)()
    nc.compile()
    return nc


def _get_nc():
    global _NC
    if _NC is None:
        _NC = _build_nc()
    return _NC


def _pack_inputs(x_b, W_qkv, b_qkv, W_proj, g):
    """Host-side packing for core (batch, head-group g): fp16, DMA-friendly."""
    f16 = np.float16
    s0 = HD * g
    xt = np.ascontiguousarray(x_b.T).astype(f16)          # [C, T]
    xqa = np.ascontiguousarray(
        xt.reshape(CC, P, 4, 512).transpose(1, 2, 0, 3)   # [p, quarter, o, t]
    )

    def wpack(col0):
        w = W_qkv[:, col0:col0 + HD].astype(f16)          # [C, HD]
        return np.ascontiguousarray(w.reshape(CC, P, 2, P).transpose(1, 2, 0, 3))

    wv_ = W_qkv[:, 2 * C + s0:2 * C + s0 + HD].astype(f16)
    wv_p = np.ascontiguousarray(wv_.reshape(CC, P, HD).transpose(1, 0, 2))
    wp_ = W_proj[s0:s0 + HD, :].astype(f16)               # [HD, C]
    wp_p = np.ascontiguousarray(wp_.reshape(2, P, C).transpose(1, 0, 2))

    cstm = np.zeros((P, CSTW), dtype=f16)
    cstm[:, OFF_TRI:OFF_TRI + P] = np.triu(np.ones((P, P), dtype=f16))
    cstm[D, OFF_SEL:OFF_SEL + D] = 1.0
    cstm[0, OFF_SEL + D:OFF_SEL + P] = 1.0
    cstm[:, OFF_BQ:OFF_BQ + 2] = b_qkv[s0:s0 + HD].reshape(2, P).T
    cstm[:, OFF_BK:OFF_BK + 2] = (
        b_qkv[C + s0:C + s0 + HD].reshape(2, P).T
    )
    cstm[:, OFF_BV:OFF_BV + HD] = b_qkv[2 * C + s0:2 * C + s0 + HD]

    return {
        "xq": xqa,
        "wq2": wpack(s0),
        "wk2": wpack(C + s0),
        "wv2": wv_p,
        "wp2": wp_p,
        "cst": np.ascontiguousarray(cstm),
    }


def kernel(x, W_qkv, b_qkv, W_proj, b_proj):
    global LAST_RESULTS
    from concourse import bass_utils

    x = np.asarray(x, dtype=np.float32)
    W_qkv = np.asarray(W_qkv, dtype=np.float32)
    b_qkv = np.asarray(b_qkv, dtype=np.float32)
    W_proj = np.asarray(W_proj, dtype=np.float32)
    b_proj = np.asarray(b_proj, dtype=np.float32)

    nc = _get_nc()
    in_maps = []
    for c in range(8):
        b, g = divmod(c, 4)
        in_maps.append(_pack_inputs(x[b], W_qkv, b_qkv, W_proj, g))

    res = bass_utils.run_bass_kernel_spmd(nc, in_maps, core_ids=list(range(8)))
    LAST_RESULTS = res
    ys = []
    for b in range(2):
        y = res.results[4 * b]["out"].astype(np.float64)
        for g in range(1, 4):
            y = y + res.results[4 * b + g]["out"]
        ys.append((y + b_proj).astype(np.float32))
    return np.stack(ys, axis=0)


# revision 14
# speedup vs baseline: 1.7471x; 1.0155x over previous
"""Causal self-attention (B=2, T=2048, C=1024, H=16) on 8 Trainium2 cores.

Sharding: data-parallel over batch (2) x tensor-parallel over heads (4 groups
of 4 heads). Core c handles batch b = c//4, head group g = c%4 (heads 4g..4g+3).
Each core computes its qkv column slice, full causal TxT attention for its 4
heads, and a partial row-parallel projection. Host sums the 4 partial proj
outputs per batch and adds b_proj.

Device kernel layout notes (v3):
- all matmul operands are fp16: on TRN2 fp32r runs fp32_mode=HIGH (2 PE
  passes -> 2 cycles/row, doubled LDWEIGHTS, and the row-bank conflict
  serializes the two K=64 head-quadrant score matmuls). fp16 is 1 cycle/row,
  enables FWL weight loads, and the hi=0/hi=1 score matmuls (stationary rows
  0-63 / 64-127) genuinely overlap (measured dstart ~4ns). PSUM stays fp32.
- feature-major ("transposed") layouts throughout: qT/kT [d, t] so PE
  contraction dims line up with no on-device transposes
- host pre-packs every DRAM tensor so each DMA moves 2-8KB contiguous
  per-partition lines (small strided lines measured ~85GB/s/queue; packed
  ~200+GB/s), and the first-needed tensors (wq col group 0, x t-quarter 0)
  are triggered first; all small constants ship as one [128, 516] f16 blob
- softmax without max-subtraction (logits ~N(0,1), exp fits fp16); exp runs
  on the ACT engine - the second binding resource (~70us) after the PE
  (~100us); the AV stationary operand embeds an all-ones column so each AV
  matmul also emits the softmax denominator row for free
- denominator rows are partition-broadcast with a single K=128 matmul against
  a constant selector matrix (col j<64 reads row 64 = den0, col j>=64 reads
  row 0 = den1) over a pre-zeroed staging row-pair; reciprocal via the
  custom-DVE reciprocal_approx_fast (~5x faster than exact reciprocal)
- causal masking: upper-triangle j-chunks skipped; diagonal chunks narrow the
  score/exp/AV column range to [o*128, 512) and one [128,128] triangular
  multiplicative mask handles the partial strip
- scheduling: the score matmul for chunk jc+1 issues before the AV for jc so
  the in-order PE stream never waits on ACT; "filler" PE work (projection of
  finished i-chunks + deferred phase-1 work: the last t-quarter of qT/kT and
  v chunks 12-15) is woven in at a fixed cadence to keep the PE at the ACT
  rate. Dense PE occupancy also parks the HAM clock gate at 2.4 GHz (any
  ~3.4us idle window re-throttles the PE to 1.2 GHz).
- normalization lags one block: den-row copies (DVE) issue right after the
  block's last AV, the broadcast-matmul + reciprocal + yT scale are emitted
  inside the NEXT block so the PE never stalls on DVE latency
- PSUM budget (8 banks): "s" tag [128,2,512] bufs=2 (scores, proj, den
  broadcast, and phase-1 qkv groups all share it) + av0/av1 bufs=2 each
- psum->sbuf copies for the projection ride the GpSimd engine (DVE is the
  #3 resource); output tiles stream out per-tile as fp16 on the sync queue
"""

import os
import sys

sys.path.insert(0, "/opt/trn_rl_repo")

import numpy as np

P = 128
T = 2048
C = 1024
D = 64
HPC = 4          # heads per core
HD = HPC * D     # 256 qkv columns per core
CC = C // P      # 8 contraction chunks
TC = T // P      # 16 t-chunks of 128
IC = T // 512    # 4 i-chunks of 512

# const blob column offsets
OFF_TRI = 0
OFF_SEL = 128
OFF_BQ = 256
OFF_BK = 258
OFF_BV = 260
CSTW = 516

_NC = None
LAST_RESULTS = None


def _build_nc():
    import concourse.mybir as mybir
    import concourse.tile as tile
    from concourse import bacc
    from contextlib import ExitStack

    dt = mybir.dt
    f32 = dt.float32
    f16 = dt.float16
    ALU = mybir.AluOpType
    ACTF = mybir.ActivationFunctionType

    nc = bacc.Bacc(
        "TRN2",
        target_bir_lowering=False,
        debug=False,
        enable_asserts=False,
        num_devices=8,
    )

    # host-packed layouts: contiguous per-partition lines per transfer
    xq = nc.dram_tensor("xq", [P, 4, CC, 512], f16, kind="ExternalInput").ap()
    wq2 = nc.dram_tensor("wq2", [P, 2, CC, P], f16, kind="ExternalInput").ap()
    wk2 = nc.dram_tensor("wk2", [P, 2, CC, P], f16, kind="ExternalInput").ap()
    wv2 = nc.dram_tensor("wv2", [P, CC, HD], f16, kind="ExternalInput").ap()
    wp2 = nc.dram_tensor("wp2", [P, 2, C], f16, kind="ExternalInput").ap()
    cst = nc.dram_tensor("cst", [P, CSTW], f16, kind="ExternalInput").ap()
    out = nc.dram_tensor("out", [T, C], f16, kind="ExternalOutput").ap()

    with tile.TileContext(nc) as tc, ExitStack() as ctx:
        persist = ctx.enter_context(tc.tile_pool(name="persist", bufs=1))
        qT_sb = persist.tile([P, 2, T], f16, name="qT")    # [d%128, dchunk, t]
        kT_sb = persist.tile([P, 2, T], f16, name="kT")
        v_sb = persist.tile([P, TC, 2, 2, P], f16, name="v")  # [t%128, tchunk, hpair, hi, 128-padded d]
        yT_sb = persist.tile([P, 2, T], f16, name="yT")
        wp_sb = persist.tile([P, 2, C], f16, name="wps")
        cst_sb = persist.tile([P, CSTW], f16, name="csts")
        dsb = persist.tile([P, 512], f16, name="dsb")
        # x is t-quarter-major and the qk weights co-major so every DMA
        # lands with 2-8KB contiguous runs on BOTH sides (small runs
        # measured ~8 B/ns per DMA engine; 4KB runs ~21 B/ns)
        xs_sb = persist.tile([P, 4, CC, 512], f16, name="xss")
        wq_sb = persist.tile([P, 2, CC, P], f16, name="wqs")
        wk_sb = persist.tile([P, 2, CC, P], f16, name="wks")
        wv_sb = persist.tile([P, CC, HD], f16, name="wvs")

        tri_v = cst_sb[:, OFF_TRI:OFF_TRI + P]
        sel_v = cst_sb[:, OFF_SEL:OFF_SEL + P]
        bq_v = cst_sb[:, OFF_BQ:OFF_BQ + 2]
        bk_v = cst_sb[:, OFF_BK:OFF_BK + 2]
        bv_v = cst_sb[:, OFF_BV:OFF_BV + HD].rearrange(
            "p (hp hi d) -> p hp hi d", hi=2, d=D
        )

        ph2 = ctx.enter_context(tc.tile_pool(name="ph2", bufs=4))
        ph3 = ctx.enter_context(tc.tile_pool(name="ph3", bufs=3))
        ps2s = ctx.enter_context(tc.tile_pool(name="ps2s", bufs=2, space="PSUM"))
        ps2a = ctx.enter_context(tc.tile_pool(name="ps2a", bufs=2, space="PSUM"))

        # one-time inits (no inputs needed)
        nc.vector.memset(dsb[:, :], 0.0)
        # constant regions of the padded v operand: zeros + the ones column
        # that makes each AV matmul emit the softmax denominator row
        nc.vector.memset(v_sb[:, :, :, 0, D + 1:P], 0.0)
        nc.vector.memset(v_sb[:, :, :, 1, 1:D], 0.0)
        nc.vector.memset(v_sb[:, :, :, 0, D:D + 1], 1.0)
        nc.vector.memset(v_sb[:, :, :, 1, 0:1], 1.0)

        # ---------------- DMA, first-needed-first ----------------
        # first x quarter rides 4 trigger queues so the first matmul group
        # (which contracts over all 8 cc chunks) unblocks as early as possible
        nc.sync.dma_start(xs_sb[:, 0, 0:3, :], xq[:, 0, 0:3, :])
        nc.gpsimd.dma_start(xs_sb[:, 0, 3:7, :], xq[:, 0, 3:7, :])
        nc.scalar.dma_start(wq_sb[:, 0, :, :], wq2[:, 0, :, :])
        nc.scalar.dma_start(xs_sb[:, 0, 7:8, :], xq[:, 0, 7:8, :])
        nc.gpsimd.dma_start(wk_sb[:, 0, :, :], wk2[:, 0, :, :])
        nc.sync.dma_start(cst_sb[:], cst)
        nc.sync.dma_start(wq_sb[:, 1, :, :], wq2[:, 1, :, :])
        nc.gpsimd.dma_start(wk_sb[:, 1, :, :], wk2[:, 1, :, :])
        nc.gpsimd.dma_start(wv_sb[:], wv2)
        for tq in range(1, 4):
            nc.sync.dma_start(xs_sb[:, tq, 0:4, :], xq[:, tq, 0:4, :])
            nc.gpsimd.dma_start(xs_sb[:, tq, 4:8, :], xq[:, tq, 4:8, :])
        nc.gpsimd.dma_start(wp_sb[:], wp2)

        # ---------------- phase 1 units ----------------
        qk_tiles = {}

        def emit_qk_half(W_s, bco, dest, co, tsl, half):
            key = (id(dest), co, tsl)
            if half == 0:
                qk_tiles[key] = ps2s.tile(
                    [P, 2, 512], f32, tag="s", name=f"qkp{co}_{tsl}"
                )
            ps = qk_tiles[key]
            for cc in range(4 * half, 4 * half + 4):
                nc.tensor.matmul(
                    ps[:, 0, :],
                    W_s[:, co, cc, :],
                    xs_sb[:, tsl, cc, :],
                    start=(cc == 0),
                    stop=(cc == CC - 1),
                )
            if half == 1:
                nc.vector.tensor_tensor(
                    dest[:, co, tsl * 512:(tsl + 1) * 512],
                    ps[:, 0, :],
                    bco.to_broadcast([P, 512]),
                    ALU.add,
                )
                del qk_tiles[key]

        def emit_qk(W_s, bco, dest, co, tsl):
            emit_qk_half(W_s, bco, dest, co, tsl, 0)
            emit_qk_half(W_s, bco, dest, co, tsl, 1)

        def emit_v(tj):
            ps = ps2s.tile([P, 2, 512], f32, tag="s")
            for cc in range(CC):
                nc.tensor.matmul(
                    ps[:, 0, 0:HD],
                    xs_sb[:, tj // 4, cc, (tj % 4) * P:(tj % 4 + 1) * P],
                    wv_sb[:, cc, :],
                    start=(cc == 0),
                    stop=(cc == CC - 1),
                )
            psv = ps[:, 0, 0:HD].rearrange("p (hp hi d) -> p hp hi d", hi=2, d=D)
            nc.vector.tensor_tensor(
                v_sb[:, tj, :, 0, 0:D], psv[:, :, 0, :], bv_v[:, :, 0, :], ALU.add
            )
            nc.vector.tensor_tensor(
                v_sb[:, tj, :, 1, D:P], psv[:, :, 1, :], bv_v[:, :, 1, :], ALU.add
            )

        # main phase 1: t-quarters 0-2 of qT/kT + v chunks 0-11. The rest
        # (quarter 3 + v 12-15) becomes attention-phase PE filler. co-outer
        # order so the second group (wk co=0, own DMA queue) never waits on
        # the wq co=1 transfer.
        for tsl in range(2):
            for co in range(2):
                for W_s, boff, dest in (
                    (wq_sb, OFF_BQ, qT_sb),
                    (wk_sb, OFF_BK, kT_sb),
                ):
                    emit_qk(
                        W_s, cst_sb[:, boff + co:boff + co + 1], dest, co, tsl
                    )
            for tj in range(4 * tsl, 4 * tsl + 4):
                emit_v(tj)

        # quarters 2-3 of phase 1 are filler inside the attention phase (the
        # PE runs ~640ns/chunk vs ACT's ~1.1us/chunk there); tagged with the
        # t-quarter so blocks that need them can force-drain first
        ph1_fill = []
        for tsl in (2, 3):
            for co in range(2):
                for W_s, boff, dest in (
                    (wk_sb, OFF_BK, kT_sb),
                    (wq_sb, OFF_BQ, qT_sb),
                ):
                    for half in range(2):
                        ph1_fill.append((tsl, "qkh", lambda
                            W_s=W_s, boff=boff, dest=dest, co=co, tsl=tsl,
                            half=half: emit_qk_half(
                                W_s, cst_sb[:, boff + co:boff + co + 1],
                                dest, co, tsl, half,
                            )))
            for tj in range(4 * tsl, 4 * tsl + 4):
                ph1_fill.append((tsl, "v", lambda tj=tj: emit_v(tj)))
        proj_fill = []

        # ---------------- phase 2: attention + woven proj ----------------
        ot_tiles = {}

        def emit_proj_half(tj, co):
            if co == 0:
                ot_tiles[tj] = ph3.tile([P, C], f16, tag="ot", name=f"ot{tj}")
            ot = ot_tiles[tj]
            pps = ps2s.tile([P, 2, 512], f32, tag="s")
            for dc in range(2):
                nc.tensor.matmul(
                    pps[:, co, :],
                    yT_sb[:, dc, tj * P:(tj + 1) * P],
                    wp_sb[:, dc, co * 512:(co + 1) * 512],
                    start=(dc == 0),
                    stop=(dc == 1),
                )
            # last i-chunk: ACT has no exp left, split the copies so the
            # tail drains twice as fast
            if tj >= 12 and co == 1:
                nc.scalar.copy(ot[:, co * 512:(co + 1) * 512], pps[:, co, :])
            else:
                nc.vector.tensor_copy(
                    ot[:, co * 512:(co + 1) * 512], pps[:, co, :]
                )
            if co == 1:
                nc.sync.dma_start(out[tj * P:(tj + 1) * P, :], ot[:])
                del ot_tiles[tj]

        # emitted-work accounting (ns) to pace fillers: the PE stream should
        # stay at least as long as the ACT (exp) stream it depends on
        clk = {"pe": 0.0, "act": 0.0}

        def fill(n):
            for _ in range(n):
                if ph1_fill:
                    q, kind, fn = ph1_fill.pop(0)
                    fn()
                    clk["pe"] += 853.0
                elif proj_fill:
                    proj_fill.pop(0)()
                    clk["pe"] += 450.0
                else:
                    return

        def fill_to_rate():
            while (ph1_fill or proj_fill) and clk["act"] > clk["pe"]:
                fill(1)

        def drain_ph1(upto_quarter):
            while ph1_fill and ph1_fill[0][0] <= upto_quarter:
                q, kind, fn = ph1_fill.pop(0)
                fn()

        def emit_norm(hp, i0, av0, av1):
            # den rows were already copied into dsb right after the block's
            # last AV; broadcast + reciprocal + scale into yT
            bps = ps2s.tile([P, 2, 512], f32, tag="s")
            nc.tensor.matmul(
                bps[:, 0, :], sel_v, dsb[:],
                start=True, stop=True, skip_group_check=True,
            )
            rec = ph2.tile([P, 512], f32, tag="rec")
            nc.vector.reciprocal_approx_fast(rec[:, :], bps[:, 0, :])
            nc.vector.tensor_tensor(
                yT_sb[0:D, hp, i0:i0 + 512], av0[0:D, :], rec[0:D, :], ALU.mult
            )
            nc.vector.tensor_tensor(
                yT_sb[D:P, hp, i0:i0 + 512], av1[D:P, :], rec[D:P, :], ALU.mult
            )
            if hp == 1:
                ci = i0 // 512
                for tj in range(4 * ci, 4 * ci + 4):
                    for co in range(2):
                        proj_fill.append(
                            lambda tj=tj, co=co: emit_proj_half(tj, co)
                        )

        pending = None
        for ci in range(IC):
            i0 = ci * 512
            njc = 4 * (ci + 1)
            if ci >= 2:
                # phase-1 stragglers must land before the blocks that read
                # them (quarter ci for the kT/qT/v this i-chunk touches)
                drain_ph1(ci)
            for hp in range(2):
                av0 = ps2a.tile([P, 512], f32, tag="av0")
                av1 = ps2a.tile([P, 512], f32, tag="av1")

                def emit_s(jc):
                    diag = jc >= 4 * ci
                    o = (jc - 4 * ci) if diag else 0
                    c0 = o * P
                    sps = ps2s.tile([P, 2, 512], f32, tag="s")
                    for hi in range(2):
                        bp = D * hi
                        nc.tensor.matmul(
                            sps[:, hi, c0:512],
                            kT_sb[bp:bp + D, hp, jc * P:(jc + 1) * P],
                            qT_sb[bp:bp + D, hp, i0 + c0:i0 + 512],
                            start=True,
                            stop=True,
                            skip_group_check=True,
                        )
                    ex = ph2.tile([P, 2, 512], f16, tag="ex")
                    nc.scalar.activation(
                        ex[:, :, c0:512],
                        sps[:, :, c0:512],
                        ACTF.Exp,
                        scale=float(D) ** -0.5,
                    )
                    if diag:
                        nc.vector.tensor_tensor(
                            ex[:, :, c0:c0 + P],
                            ex[:, :, c0:c0 + P],
                            tri_v[:, None, :].to_broadcast([P, 2, P]),
                            ALU.mult,
                        )
                    return ex, c0

                def emit_av(jc, ex, c0):
                    for hi, av in ((0, av0), (1, av1)):
                        nc.tensor.matmul(
                            av[:, c0:512],
                            v_sb[:, jc, hp, hi, :],
                            ex[:, hi, c0:512],
                            start=(jc == 0),
                            stop=(jc == njc - 1),
                            skip_group_check=True,
                        )

                # score jc+1 issues before AV jc so the in-order PE stream
                # never waits on the ACT exp; fillers pad the PE to ACT rate;
                # the previous block's normalization lands at jc==3, behind a
                # filler, so its broadcast-matmul never waits on the den rows
                pend_av = None
                for jc in range(njc):
                    ex, c0 = emit_s(jc)
                    w = 512 - (jc - 4 * ci) * P if jc >= 4 * ci else 512
                    clk["pe"] += w / 2.4 + 8
                    clk["act"] += 2 * w / 1.2 + 160
                    # fillers go between the score and the AV that depends on
                    # the previous chunk's exp, absorbing ACT-rate jitter
                    fill_to_rate()
                    if pend_av is not None:
                        emit_av(*pend_av)
                        clk["pe"] += 2 * (512 - pend_av[2]) / 2.4 + 16
                    pend_av = (jc, ex, c0)
                    if jc == 3 and pending is not None:
                        emit_norm(*pending)
                        pending = None
                        clk["pe"] += 230.0
                emit_av(*pend_av)
                clk["pe"] += 2 * (512 - pend_av[2]) / 2.4 + 16
                # den rows -> staging now; the rest of the normalization is
                # emitted inside the next block so the PE never stalls on it
                nc.vector.tensor_copy(dsb[D:D + 1, :], av0[D:D + 1, :])
                nc.vector.tensor_copy(dsb[0:1, :], av1[0:1, :])
                pending = (hp, i0, av0, av1)
        fill(1)
        emit_norm(*pending)
        while proj_fill:
            proj_fill.pop(0)()
    nc.compile()
    return nc


def _get_nc():
    global _NC
    if _NC is None:
        _NC = _build_nc()
    return _NC


def _pack_inputs(x_b, W_qkv, b_qkv, W_proj, g):
    """Host-side packing for core (batch, head-group g): fp16, DMA-friendly."""
    f16 = np.float16
    s0 = HD * g
    xt = np.ascontiguousarray(x_b.T).astype(f16)          # [C, T]
    xqa = np.ascontiguousarray(
        xt.reshape(CC, P, 4, 512).transpose(1, 2, 0, 3)   # [p, quarter, o, t]
    )

    def wpack(col0):
        w = W_qkv[:, col0:col0 + HD].astype(f16)          # [C, HD]
        return np.ascontiguousarray(w.reshape(CC, P, 2, P).transpose(1, 2, 0, 3))

    wv_ = W_qkv[:, 2 * C + s0:2 * C + s0 + HD].astype(f16)
    wv_p = np.ascontiguousarray(wv_.reshape(CC, P, HD).transpose(1, 0, 2))
    wp_ = W_proj[s0:s0 + HD, :].astype(f16)               # [HD, C]
    wp_p = np.ascontiguousarray(wp_.reshape(2, P, C).transpose(1, 0, 2))

    cstm = np.zeros((P, CSTW), dtype=f16)
    cstm[:, OFF_TRI:OFF_TRI + P] = np.triu(np.ones((P, P), dtype=f16))
    cstm[D, OFF_SEL:OFF_SEL + D] = 1.0
    cstm[0, OFF_SEL + D:OFF_SEL + P] = 1.0
    cstm[:, OFF_BQ:OFF_BQ + 2] = b_qkv[s0:s0 + HD].reshape(2, P).T
    cstm[:, OFF_BK:OFF_BK + 2] = (
        b_qkv[C + s0:C + s0 + HD].reshape(2, P).T
    )
    cstm[:, OFF_BV:OFF_BV + HD] = b_qkv[2 * C + s0:2 * C + s0 + HD]

    return {
        "xq": xqa,
        "wq2": wpack(s0),
        "wk2": wpack(C + s0),
        "wv2": wv_p,
        "wp2": wp_p,
        "cst": np.ascontiguousarray(cstm),
    }


def kernel(x, W_qkv, b_qkv, W_proj, b_proj):
    global LAST_RESULTS
    from concourse import bass_utils

    x = np.asarray(x, dtype=np.float32)
    W_qkv = np.asarray(W_qkv, dtype=np.float32)
    b_qkv = np.asarray(b_qkv, dtype=np.float32)
    W_proj = np.asarray(W_proj, dtype=np.float32)
    b_proj = np.asarray(b_proj, dtype=np.float32)

    nc = _get_nc()
    in_maps = []
    for c in range(8):
        b, g = divmod(c, 4)
        in_maps.append(_pack_inputs(x[b], W_qkv, b_qkv, W_proj, g))

    res = bass_utils.run_bass_kernel_spmd(nc, in_maps, core_ids=list(range(8)))
    LAST_RESULTS = res
    ys = []
    for b in range(2):
        y = res.results[4 * b]["out"].astype(np.float64)
        for g in range(1, 4):
            y = y + res.results[4 * b + g]["out"]
        ys.append((y + b_proj).astype(np.float32))
    return np.stack(ys, axis=0)
